# revision 19
# baseline (speedup 1.0000x reference)
import os
import sys

sys.path.insert(0, "/opt/trn_rl_repo")

import numpy as np

# ---------------------------------------------------------------- problem dims
NCORES = 8
N = 50000
E = 800000
IN_F, HID_F, OUT_F = 256, 128, 64
NEG = 0.2
EPS = 1e-16

NPC = N // NCORES            # 6250 target nodes per core
BPB = 32                     # targets per block (one-hot width)
NB = (NPC + BPB - 1) // BPB  # 196 blocks per core
GPB = 4                      # blocks per group (PSUM packs 4x32 targets)
NG = NB // GPB               # 49 groups
ROWS = NB * BPB              # 6272 padded rows per core slice
TBL = NCORES * ROWS          # 50176 rows in the all-gathered table
TH = 32768                   # int16 gather index threshold
WSHIFT = 8.0                 # global exp shift (cancels in normalization)
NW1 = 67                     # L1 aggregated width: v(64) + u2pre + t2pre + 1
NW2 = 65                     # L2 aggregated width: h2(64) + 1


def prep_structures(edge_index):
    """Host-side layout of the edge list.

    Slot storage per group g (4 blocks of 32 targets):
      [lo slots of b0..b3 | hi slots of b0..b3]
    so each group needs exactly TWO dma_gather calls (one per table base).
    nlo_b / nhi_b are uniform across cores (compile-time structure); unused
    idx positions hold dummy index 0 with REL=-1 (contributes nothing), so
    num_idxs_reg == num_idxs uniformly across cores.
    Returns meta + per-core eidx [128, TOTCOLS] int16 and REL [128, S_TOT].
    """
    src = edge_index[0].astype(np.int64)
    tgt = edge_index[1].astype(np.int64)
    adj = (src // NPC) * ROWS + (src % NPC)  # row in all-gathered table

    order = np.argsort(tgt, kind="stable")
    src_a = adj[order]
    tgt_s = tgt[order]

    core_of = tgt_s // NPC
    tc = tgt_s % NPC
    blk_of = tc // BPB
    rel_of = tc % BPB
    gb = core_of * NB + blk_of
    bounds = np.searchsorted(gb, np.arange(NCORES * NB + 1))

    # per (core, block): lo/hi (adj, rel) lists sorted by adj
    per_kb = {}
    lo_cnt = np.zeros((NCORES, NB), dtype=np.int64)
    hi_cnt = np.zeros((NCORES, NB), dtype=np.int64)
    for k in range(NCORES):
        for b in range(NB):
            s, e = bounds[k * NB + b], bounds[k * NB + b + 1]
            sa = src_a[s:e]
            rl = rel_of[s:e]
            m = sa < TH
            lo_o = np.argsort(sa[m], kind="stable")
            hi_o = np.argsort(sa[~m], kind="stable")
            per_kb[(k, b)] = (sa[m][lo_o], rl[m][lo_o], sa[~m][hi_o], rl[~m][hi_o])
            lo_cnt[k, b] = int(m.sum())
            hi_cnt[k, b] = int((~m).sum())

    # uniform-across-cores slot counts per block
    nlo = (lo_cnt.max(axis=0) + 127) // 128    # [NB]
    nhi = (hi_cnt.max(axis=0) + 127) // 128
    NLO_g = np.array([nlo[g * GPB:(g + 1) * GPB].sum() for g in range(NG)])
    NHI_g = np.array([nhi[g * GPB:(g + 1) * GPB].sum() for g in range(NG)])
    S_g = NLO_g + NHI_g
    gs_off = np.concatenate([[0], np.cumsum(S_g)])
    S_TOT = int(gs_off[-1])
    SMAX = int(S_g.max())

    REL = np.full((NCORES, 128, S_TOT), -1.0, dtype=np.float32)

    # segments[g][q] = (lo_s0, nlo_q, hi_s0, nhi_q) slot offsets within group
    segments = []
    for g in range(NG):
        segs = []
        lo_c = 0
        hi_c = int(NLO_g[g])
        for q in range(GPB):
            b = g * GPB + q
            segs.append((lo_c, int(nlo[b]), hi_c, int(nhi[b])))
            lo_c += int(nlo[b])
            hi_c += int(nhi[b])
        segments.append(segs)

    # calls: per group: lo call then hi call
    idx_parts = [[] for _ in range(NCORES)]
    calls = []
    col_off = 0
    for g in range(NG):
        for is_hi in (False, True):
            n_slots = int((NHI_g if is_hi else NLO_g)[g])
            base_s = int(gs_off[g]) + (int(NLO_g[g]) if is_hi else 0)
            n_idx = n_slots * 128
            cols = n_idx // 16
            calls.append((col_off, cols, n_idx, base_s, n_slots, is_hi, g))
            col_off += cols
            for k in range(NCORES):
                vals = np.zeros(n_idx, dtype=np.int64)
                rr = np.full(n_idx, -1.0, dtype=np.float32)
                pos = 0
                for q in range(GPB):
                    b = g * GPB + q
                    la, lr, ha, hr = per_kb[(k, b)]
                    arr, rl = (ha, hr) if is_hi else (la, lr)
                    seg_slots = int((nhi if is_hi else nlo)[b])
                    nr = len(arr)
                    vals[pos:pos + nr] = arr - (TH if is_hi else 0)
                    rr[pos:pos + nr] = rl
                    pos += seg_slots * 128
                REL[k][:, base_s:base_s + n_slots] = (
                    rr.reshape(n_slots, 128).T
                )
                w16 = vals.reshape(-1, 16).T  # [16, cols]
                idx_parts[k].append(np.tile(w16, (8, 1)).astype(np.int16))
    eidx = [np.concatenate(idx_parts[k], axis=1) for k in range(NCORES)]

    gcalls = [[] for _ in range(NG)]
    for ci, c in enumerate(calls):
        gcalls[c[6]].append(ci)
    gcol = []
    for g in range(NG):
        cs_ = [calls[ci] for ci in gcalls[g]]
        lo_c = min(c[0] for c in cs_)
        hi_c = max(c[0] + c[1] for c in cs_)
        gcol.append((lo_c, hi_c))

    meta = dict(
        S_g=S_g, gs_off=gs_off, S_TOT=S_TOT, SMAX=SMAX,
        calls=calls, gcalls=gcalls, gcol=gcol, TOTCOLS=col_off,
        nlo=nlo, nhi=nhi, segments=segments,
    )
    return meta, eidx, REL


# ------------------------------------------------------------------ host model
def host_model(inputs, f16=True):
    """Numpy mirror of the device dataflow (for algorithm validation)."""
    x = np.asarray(inputs["x"], np.float32)
    ei = np.asarray(inputs["edge_index"])
    W1 = np.asarray(inputs["W1"], np.float32)
    b1 = np.asarray(inputs["b1"], np.float32)
    a1w = np.asarray(inputs["a1_w"], np.float32)
    a1b = np.asarray(inputs["a1_b"], np.float32)
    W2 = np.asarray(inputs["W2"], np.float32)
    b2 = np.asarray(inputs["b2"], np.float32)
    a2w = np.asarray(inputs["a2_w"], np.float32)
    a2b = np.asarray(inputs["a2_b"], np.float32)

    meta, eidx, REL = prep_structures(ei)
    ed = np.float16 if f16 else np.float32
    a1h, a1t = a1w[:HID_F], a1w[HID_F:]
    a2h, a2t = a2w[:OUT_F], a2w[OUT_F:]
    w2a2h = W2 @ a2h
    w2a2t = W2 @ a2t
    u2bias = float(a2h @ b2)
    t2bias = float(a2t @ b2 + a2b[0])

    # ---- phase 1: per-core table1 rows [u1, v(64), u2pre, t2pre, 1, junk]
    t1_sl, tbl_sl = [], []
    for k in range(NCORES):
        xs = np.zeros((ROWS, IN_F), np.float32)
        xs[:NPC] = x[k * NPC:(k + 1) * NPC]
        h = xs @ W1 + b1
        h = np.where(h > 0, h, np.expm1(np.minimum(h, 0.0)))
        row = np.zeros((ROWS, 128), np.float32)
        row[:, 0] = h @ a1h
        row[:, 1:65] = h @ W2
        row[:, 65] = h @ w2a2h
        row[:, 66] = h @ w2a2t
        row[:, 67] = 1.0
        t1_sl.append((h @ a1t + a1b[0]).astype(np.float32))
        tbl_sl.append(row.astype(ed))
    table1 = np.concatenate(tbl_sl, axis=0)

    def edge_phase(k, table, t_sl, nw):
        """Returns acc [ROWS, nw] (unnormalized sums + denominator)."""
        acc = np.zeros((ROWS, nw), np.float32)
        relk = REL[k]
        gs_off = meta["gs_off"]
        for g in range(NG):
            S = int(meta["S_g"][g])
            so = int(gs_off[g])
            # gather (all idx positions valid; dummies point at row 0)
            gt = np.zeros((128, S, 128), ed)
            for ci in meta["gcalls"][g]:
                c0, cols, n_idx, base_s, n_slots, is_hi, _g = meta["calls"][ci]
                if n_slots == 0:
                    continue
                w16 = eidx[k][:16, c0:c0 + cols]
                flat = w16.T.flatten()[:n_idx].astype(np.int64)
                flat = flat + (TH if is_hi else 0)
                rows = table[flat].astype(ed)
                gt[:, base_s - so:base_s - so + n_slots, :] = np.transpose(
                    rows.reshape(n_slots, 128, 128), (1, 0, 2)
                )
            trow = np.zeros((NG * 128,), np.float32)
            trow[:ROWS] = t_sl
            trowg = trow[g * 128:(g + 1) * 128].astype(ed)  # [128]
            rel_g = relk[:, so:so + S]  # [128, S]
            iota = np.arange(BPB, dtype=np.float32)
            Mx = (rel_g[:, :, None] == iota[None, None, :]).astype(ed)  # [128,S,32]
            # tsel: per (q, segment)
            tsel = np.zeros((128, S, BPB), ed)
            for q in range(GPB):
                tw = trowg[q * BPB:(q + 1) * BPB]
                lo_s0, nlo_q, hi_s0, nhi_q = meta["segments"][g][q]
                for (s0, ns_) in ((lo_s0, nlo_q), (hi_s0, nhi_q)):
                    tsel[:, s0:s0 + ns_, :] = (
                        Mx[:, s0:s0 + ns_, :] * tw[None, None, :]
                    )
            ted = tsel.sum(axis=2, dtype=np.float32).astype(ed)  # [128, S]
            z = (gt[:, :, 0].astype(np.float32) + ted.astype(np.float32))
            zl = np.maximum(z, NEG * z)
            w = np.exp(zl - WSHIFT).astype(ed)
            Wm = (Mx * w[:, :, None]).astype(ed)  # [128, S, 32]
            for q in range(GPB):
                r0 = g * 128 + q * BPB
                a = np.zeros((BPB, nw), np.float32)
                lo_s0, nlo_q, hi_s0, nhi_q = meta["segments"][g][q]
                slots = list(range(lo_s0, lo_s0 + nlo_q)) + list(
                    range(hi_s0, hi_s0 + nhi_q)
                )
                for s in slots:
                    a += (
                        Wm[:, s, :].astype(np.float32).T
                        @ gt[:, s, 1:1 + nw].astype(np.float32)
                    )
                acc[r0:r0 + BPB] = a
        return acc

    # ---- L1 edge phase + L2 table build
    t2_sl, tbl2_sl = [], []
    for k in range(NCORES):
        acc = edge_phase(k, table1, t1_sl[k], NW1)
        den = acc[:, 66:67] + EPS
        h2 = acc[:, 0:64] / den + b2
        u2 = acc[:, 64] / den[:, 0] + u2bias
        t2 = acc[:, 65] / den[:, 0] + t2bias
        row = np.zeros((ROWS, 128), np.float32)
        row[:, 0] = u2
        row[:, 1:65] = h2
        row[:, 65] = 1.0
        t2_sl.append(t2.astype(np.float32))
        tbl2_sl.append(row.astype(ed))
    table2 = np.concatenate(tbl2_sl, axis=0)
    host_model.table1 = table1
    host_model.table2 = table2
    host_model.t1_sl = t1_sl
    host_model.t2_sl = t2_sl

    outs = []
    for k in range(NCORES):
        acc = edge_phase(k, table2, t2_sl[k], NW2)
        den = acc[:, 64:65] + EPS
        o2 = acc[:, 0:64] / den
        m = o2.max(axis=1, keepdims=True)
        lse = np.log(np.exp(o2 - m).sum(axis=1, keepdims=True)) + m
        outs.append((o2 - lse)[:NPC])
    return np.concatenate(outs, axis=0).astype(np.float32)


if __name__ == "__main__":
    sys.path.insert(0, os.path.dirname(os.path.abspath(__file__)))
    import reference

    inputs = {k: np.asarray(v) for k, v in reference.setup_inputs().items()}
    expect = np.asarray(reference.reference(**inputs))
    for f16 in (True, False):
        got = host_model(inputs, f16=f16)
        err = np.abs(got - expect)
        rel = err.max() / np.abs(expect).max()
        print(f"host_model f16={f16}: absmax {err.max():.3e} rel {rel:.3e}")


# ------------------------------------------------------------------ bass build
def _patch_tile_drain():
    """This walrus build supports only one sync-wait per SP TPB_CTRL
    instruction; split TileContext's exit drain into single-wait NOPs."""
    import concourse.mybir as mybir
    import concourse.tile as tile
    from concourse.tile import ScopedClock

    if getattr(tile.TileContext, "_drain_split_patched", False):
        return

    def _split(self, tick_clock, wait_clock):
        nop0 = self.nc.sync.nop()
        wait_clock.add_sem_waits(
            nop0.ins, ScopedClock({None: tick_clock.global_clock})
        )
        si = nop0.ins.sync_info
        if si is not None and si.on_wait and len(si.on_wait) > 1:
            waits = list(si.on_wait)
            nop0.ins.sync_info = mybir.SyncInfo(
                on_wait=[waits[0]], on_update=list(si.on_update)
            )
            for w in waits[1:]:
                n = self.nc.sync.nop()
                n.ins.sync_info = mybir.SyncInfo(on_wait=[w], on_update=[])
        self.nc.sync.drain()
        self.nc.all_engine_barrier()
        popped = self.nc._tile_sem_poison_stack.pop()
        assert popped is self._sem_poison
        self.nc.clear_and_free_semaphores(list(self.sems.allocated().values()))
        self.nc.all_engine_barrier()

    tile.TileContext._drain_and_barrier = _split
    tile.TileContext._drain_split_patched = True


def _split_multi_waits(nc):
    """Move extra sync waits onto single-wait NOPs (walrus build limit)."""
    import concourse.mybir as mybir

    ctr = [0]
    for f in nc.m.functions:
        for bb in f.blocks:
            insts = list(bb.instructions)
            out = []
            changed = False
            for ins in insts:
                si = getattr(ins, "sync_info", None)
                if si is not None and si.on_wait and len(si.on_wait) > 1:
                    waits = list(si.on_wait)
                    for w in waits[:-1]:
                        n = mybir.InstNoOp(
                            name=f"splitw-{ctr[0]}", ins=[], outs=[]
                        )
                        ctr[0] += 1
                        n.engine = ins.engine
                        n.sync_info = mybir.SyncInfo(on_wait=[w], on_update=[])
                        nc.register_instruction(n)
                        out.append(n)
                    ins.sync_info = mybir.SyncInfo(
                        on_wait=[waits[-1]], on_update=list(si.on_update)
                    )
                    changed = True
                out.append(ins)
            if changed:
                bb.instructions = out


def build_bass(meta, consts, f16=True):
    import concourse.bass as bass
    import concourse.mybir as mybir
    import concourse.tile as tile
    from concourse.library_config import mlp as mlp_lib
    from concourse.tile_rust import add_dep_helper

    _patch_tile_drain()

    F32 = mybir.dt.float32
    F16 = mybir.dt.float16 if f16 else mybir.dt.float32
    I16 = mybir.dt.int16
    AL = mybir.AluOpType
    AF = mybir.ActivationFunctionType
    AX = mybir.AxisListType

    S_TOT = meta["S_TOT"]
    SMAX = meta["SMAX"]
    gs_off = meta["gs_off"]
    gcol = meta["gcol"]
    segments = meta["segments"]
    ECOLS = max(c1 - c0 for (c0, c1) in gcol)

    nc = bass.Bass(
        num_devices=NCORES, num_swdge_queues=4,
        dynamic_dma_scratch_size=32768,
    )

    x_slT = nc.dram_tensor("x_slT", [IN_F, ROWS], F32, kind="ExternalInput")
    eidx = nc.dram_tensor("eidx", [128, meta["TOTCOLS"]], I16, kind="ExternalInput")
    tgtrel = nc.dram_tensor("tgtrel", [128, S_TOT], F16, kind="ExternalInput")
    out_fin = nc.dram_tensor("out_fin", [ROWS, OUT_F], F32, kind="ExternalOutput")

    def inl(name, arr):
        return nc.inline_tensor(np.ascontiguousarray(arr), name=name)

    np_ed = np.float16 if f16 else np.float32
    c_W1a = inl("W1a", consts["W1"][:128].astype(np.float32))
    c_W1b = inl("W1b", consts["W1"][128:].astype(np.float32))
    c_W2 = inl("W2c", consts["W2"].astype(np.float32))
    c_a1h = inl("a1h", consts["a1h"].reshape(HID_F, 1).astype(np.float32))
    c_a1t = inl("a1t", consts["a1t"].reshape(HID_F, 1).astype(np.float32))
    c_w2a2h = inl("w2a2h", consts["w2a2h"].reshape(HID_F, 1).astype(np.float32))
    c_w2a2t = inl("w2a2t", consts["w2a2t"].reshape(HID_F, 1).astype(np.float32))
    c_b1 = inl("b1c", consts["b1"].reshape(HID_F, 1).astype(np.float32))
    c_b1n = inl("b1n", (-consts["b1"]).reshape(HID_F, 1).astype(np.float32))
    c_b2bc = inl("b2bc", np.tile(consts["b2"].astype(np.float32), (128, 1)))
    c_iota = inl("iota32", np.tile(np.arange(BPB, dtype=np_ed), (128, 1)))
    c_ones1 = inl("ones1", np.ones((1, 128), np.float32))
    c_I128 = inl("I128", np.eye(128, dtype=np.float32))
    c_b1a = inl("b1a", np.full((1, 1), consts["a1b"], np.float32))
    c_nsh = inl("nsh", np.full((128, 1), -WSHIFT, np.float32))
    c_u2b = inl("u2b", np.full((128, 1), consts["u2bias"], np.float32))
    c_t2b = inl("t2b", np.full((128, 1), consts["t2bias"], np.float32))

    h1p_sl = nc.dram_tensor("h1p_sl", [ROWS, 128], F16)
    h1p_all = nc.dram_tensor("h1p_all", [TBL, 128], F16, addr_space="Shared")
    h2p_sl = nc.dram_tensor("h2p_sl", [ROWS, 128], F16)
    h2p_all = nc.dram_tensor("h2p_all", [TBL, 128], F16, addr_space="Shared")

    with tile.TileContext(nc) as tc:
        import contextlib

        with contextlib.ExitStack() as ctx:
            cpool = ctx.enter_context(tc.tile_pool(name="consts", bufs=1))
            persist = ctx.enter_context(tc.tile_pool(name="persist", bufs=1))
            sb = ctx.enter_context(tc.tile_pool(name="sb", bufs=4))
            gp = ctx.enter_context(tc.tile_pool(name="gp", bufs=4))
            ps = ctx.enter_context(tc.tile_pool(name="ps", bufs=4, space="PSUM"))
            psa = ctx.enter_context(tc.tile_pool(name="psa", bufs=2, space="PSUM"))

            def cload(handle, shape, dtype):
                t = cpool.tile(shape, dtype, tag=handle.name)
                nc.sync.dma_start(out=t[:], in_=handle[:, :])
                return t

            W1a = cload(c_W1a, [128, 128], F32)
            W1b = cload(c_W1b, [128, 128], F32)
            W2 = cload(c_W2, [128, 64], F32)
            a1h = cload(c_a1h, [128, 1], F32)
            a1t = cload(c_a1t, [128, 1], F32)
            w2a2h = cload(c_w2a2h, [128, 1], F32)
            w2a2t = cload(c_w2a2t, [128, 1], F32)
            b1c = cload(c_b1, [128, 1], F32)
            b1n = cload(c_b1n, [128, 1], F32)
            b2bc = cload(c_b2bc, [128, 64], F32)
            iota = cload(c_iota, [128, BPB], F16)
            ones1 = cload(c_ones1, [1, 128], F32)
            I128 = cload(c_I128, [128, 128], F32)
            b1a = cload(c_b1a, [1, 1], F32)
            nsh = cload(c_nsh, [128, 1], F32)
            u2b = cload(c_u2b, [128, 1], F32)
            t2b = cload(c_t2b, [128, 1], F32)

            t1_sb = persist.tile([1, ROWS], F32, tag="t1")
            t2_sb = persist.tile([1, ROWS], F32, tag="t2")
            trowall = persist.tile([128, NG * 128], F16, tag="trowall")
            trb = persist.tile([128, S_TOT], F16, tag="trb")
            nc.sync.dma_start(out=trb[:], in_=tgtrel[:, :])

            ll = nc.gpsimd.load_library(mlp_lib)
            nidx_regs = {}
            for c in meta["calls"]:
                ni = c[2]
                if ni not in nidx_regs and ni > 0:
                    r = nc.gpsimd.alloc_register(f"nidx_{ni}")
                    nc.gpsimd.reg_mov(r, ni)
                    nidx_regs[ni] = r

            # warm up gather tiles so pad columns never hold NaN bit patterns
            for _ in range(4):
                gt = gp.tile([128, SMAX, 128], F16, tag="g")
                nc.vector.memset(gt[:], 0.0)

            # ---------------- phase 1: table1 rows + t1 ----------------
            for cix in range(NG):
                r0 = cix * 128
                xT0 = sb.tile([128, 128], F32, tag="xT0")
                xT1 = sb.tile([128, 128], F32, tag="xT1")
                nc.sync.dma_start(out=xT0[:], in_=x_slT[0:128, r0:r0 + 128])
                nc.sync.dma_start(out=xT1[:], in_=x_slT[128:256, r0:r0 + 128])
                hTp = ps.tile([128, 128], F32, tag="pp")
                nc.tensor.matmul(hTp[:], lhsT=W1a[:], rhs=xT0[:], start=True, stop=False)
                nc.tensor.matmul(hTp[:], lhsT=W1b[:], rhs=xT1[:], start=False, stop=True)
                ha = sb.tile([128, 128], F32, tag="ha")
                nc.scalar.activation(ha[:], hTp[:], AF.Relu, bias=b1c[:])
                hcn = sb.tile([128, 128], F32, tag="hcn")
                nc.scalar.activation(hcn[:], hTp[:], AF.Relu, bias=b1n[:], scale=-1.0)
                hdx = sb.tile([128, 128], F32, tag="hdx")
                nc.scalar.activation(hdx[:], hcn[:], AF.Exp, scale=-1.0)
                h1T = sb.tile([128, 128], F32, tag="h1T")
                nc.vector.scalar_tensor_tensor(
                    out=h1T[:], in0=hdx[:], scalar=-1.0, in1=ha[:],
                    op0=AL.add, op1=AL.add,
                )
                P = ps.tile([128, NW1], F32, tag="pp")
                nc.tensor.matmul(P[:, 1:65], lhsT=h1T[:], rhs=W2[:], start=True, stop=True)
                nc.tensor.matmul(P[:, 0:1], lhsT=h1T[:], rhs=a1h[:], start=True, stop=True)
                nc.tensor.matmul(P[:, 65:66], lhsT=h1T[:], rhs=w2a2h[:], start=True, stop=True)
                nc.tensor.matmul(P[:, 66:67], lhsT=h1T[:], rhs=w2a2t[:], start=True, stop=True)
                t1p = ps.tile([1, 128], F32, tag="pp")
                nc.tensor.matmul(t1p[:], lhsT=a1t[:], rhs=h1T[:], start=True, stop=True)
                nc.scalar.activation(
                    t1_sb[0:1, r0:r0 + 128], t1p[:], AF.Identity, bias=b1a[:]
                )
                hrow = sb.tile([128, 128], F16, tag="hrow")
                nc.scalar.copy(hrow[:, 0:NW1], P[:])
                nc.vector.memset(hrow[:, NW1:NW1 + 1], 1.0)
                nc.sync.dma_start(out=h1p_sl[r0:r0 + 128, :], in_=hrow[:])

            nc.gpsimd.collective_compute(
                "AllGather",
                AL.bypass,
                replica_groups=[list(range(NCORES))],
                ins=[h1p_sl.ap().opt()],
                outs=[h1p_all.ap().opt()],
            )

            qctr = [0]

            def edge_layer(layer):
                if layer == 1:
                    table, t_sb, nw = h1p_all, t1_sb, NW1
                else:
                    table, t_sb, nw = h2p_all, t2_sb, NW2

                # trowall: broadcast t values down partitions, fp16
                for g in range(NG):
                    r0 = g * 128
                    trp = ps.tile([128, 128], F32, tag="pp")
                    nc.tensor.matmul(
                        trp[:], lhsT=ones1[:], rhs=t_sb[0:1, r0:r0 + 128],
                        start=True, stop=True,
                    )
                    nc.scalar.copy(trowall[:, r0:r0 + 128], trp[:])

                for g in range(NG):
                    S = int(meta["S_g"][g])
                    so = int(gs_off[g])
                    c0, c1 = gcol[g]
                    eib = sb.tile([128, ECOLS], I16, tag="eib")
                    nc.sync.dma_start(out=eib[:, 0:c1 - c0], in_=eidx[:, c0:c1])
                    gt = gp.tile([128, SMAX, 128], F16, tag="g")
                    for ci in meta["gcalls"][g]:
                        (co, cols, n_idx, base_s, n_slots, is_hi, _g) = \
                            meta["calls"][ci]
                        if n_slots == 0:
                            continue
                        tbl_ap = table[TH:TBL, :] if is_hi else table[:, :]
                        qctr[0] += 1
                        gi = nc.gpsimd.dma_gather(
                            gt[:, base_s - so:base_s - so + n_slots, :],
                            tbl_ap,
                            eib[:, co - c0:co - c0 + cols],
                            num_idxs=n_idx,
                            num_idxs_reg=nidx_regs[n_idx],
                            elem_size=128,
                            single_packet=False,
                            queue_num=qctr[0] % 4,
                        )
                        add_dep_helper(gi.ins, ll.ins)

                    M = sb.tile([128, SMAX, BPB], F16, tag="M")
                    nc.vector.tensor_tensor(
                        out=M[:, 0:S, :],
                        in0=trb[:, so:so + S].to_broadcast([128, S, BPB]),
                        in1=bass.AP(
                            iota[:].tensor, iota[:].offset,
                            [list(iota[:].ap[0]), [0, S], list(iota[:].ap[1])],
                        ),
                        op=AL.is_equal,
                    )
                    tsel = sb.tile([128, SMAX, BPB], F16, tag="tsel")
                    trg = trowall[:, g * 128:(g + 1) * 128]
                    for q in range(GPB):
                        tq = trg[:, q * BPB:(q + 1) * BPB]
                        lo_s0, nlo_q, hi_s0, nhi_q = segments[g][q]
                        for (s0_, ns_) in ((lo_s0, nlo_q), (hi_s0, nhi_q)):
                            if ns_ == 0:
                                continue
                            nc.vector.tensor_tensor(
                                out=tsel[:, s0_:s0_ + ns_, :],
                                in0=M[:, s0_:s0_ + ns_, :],
                                in1=bass.AP(
                                    tq.tensor, tq.offset,
                                    [list(tq.ap[0]), [0, ns_], list(tq.ap[1])],
                                ),
                                op=AL.mult,
                            )
                    ted = sb.tile([128, SMAX], F16, tag="ted")
                    with nc.allow_low_precision(
                        reason="one nonzero per segment; fp16 exact"
                    ):
                        nc.vector.tensor_reduce(
                            out=ted[:, 0:S], in_=tsel[:, 0:S, :],
                            axis=AX.X, op=AL.add,
                        )
                    # z = g0 + ted ; zl = max(z, 0.2 z) ; w = exp(zl - 8)
                    g0 = bass.AP(
                        gt[:].tensor, gt[:].offset,
                        [list(gt[:].ap[0]), [128, S]],
                    )
                    z = sb.tile([128, SMAX], F16, tag="z")
                    nc.vector.tensor_tensor(
                        out=z[:, 0:S], in0=g0, in1=ted[:, 0:S], op=AL.add
                    )
                    zl = sb.tile([128, SMAX], F16, tag="zl")
                    nc.vector.scalar_tensor_tensor(
                        out=zl[:, 0:S], in0=z[:, 0:S], scalar=NEG,
                        in1=z[:, 0:S], op0=AL.mult, op1=AL.max,
                    )
                    w = sb.tile([128, SMAX], F16, tag="w")
                    nc.scalar.activation(w[:, 0:S], zl[:, 0:S], AF.Exp, bias=nsh[:])
                    Wm = sb.tile([128, SMAX, BPB], F16, tag="Wm")
                    nc.vector.tensor_tensor(
                        out=Wm[:, 0:S, :],
                        in0=M[:, 0:S, :],
                        in1=w[:, 0:S].to_broadcast([128, S, BPB]),
                        op=AL.mult,
                    )

                    acc01 = psa.tile([64, nw], F32, tag="acc01")
                    acc23 = psa.tile([64, nw], F32, tag="acc23")
                    accs = [acc01, acc23]
                    for q in range(GPB):
                        acc = accs[q // 2]
                        qq = q % 2
                        lo_s0, nlo_q, hi_s0, nhi_q = segments[g][q]
                        slots = list(range(lo_s0, lo_s0 + nlo_q)) + list(
                            range(hi_s0, hi_s0 + nhi_q)
                        )
                        for si, sl in enumerate(slots):
                            nc.tensor.matmul(
                                acc[qq * BPB:(qq + 1) * BPB, :],
                                lhsT=Wm[:, sl, :],
                                rhs=gt[:, sl, 1:1 + nw],
                                start=(si == 0), stop=(si == len(slots) - 1),
                                skip_group_check=True,
                            )

                    for half, acc in enumerate(accs):
                        r0 = g * 128 + half * 64
                        den = sb.tile([64, 1], F32, tag="den")
                        nc.vector.tensor_scalar_add(den[:], acc[:, nw - 1:nw], EPS)
                        rec = sb.tile([64, 1], F32, tag="rec")
                        nc.vector.reciprocal(rec[:], den[:])
                        if layer == 1:
                            h2p = sb.tile([64, 64], F32, tag="h2p")
                            nc.vector.tensor_tensor(
                                out=h2p[:], in0=acc[:, 0:64],
                                in1=rec[:].to_broadcast([64, 64]), op=AL.mult,
                            )
                            h2 = sb.tile([64, 64], F32, tag="h2")
                            nc.vector.tensor_tensor(
                                out=h2[:], in0=h2p[:], in1=b2bc[0:64, :], op=AL.add
                            )
                            hrow2 = sb.tile([64, 128], F16, tag="hrow2")
                            nc.scalar.activation(
                                hrow2[:, 0:1], acc[:, 64:65], AF.Identity,
                                scale=rec[:], bias=u2b[0:64, :],
                            )
                            nc.scalar.copy(hrow2[:, 1:65], h2[:])
                            nc.vector.memset(hrow2[:, 65:66], 1.0)
                            t2c = sb.tile([64, 1], F32, tag="t2c")
                            nc.scalar.activation(
                                t2c[:], acc[:, 65:66], AF.Identity,
                                scale=rec[:], bias=t2b[0:64, :],
                            )
                            t2pp = ps.tile([1, 64], F32, tag="pp")
                            nc.tensor.transpose(t2pp[:], t2c[:], I128[0:64, 0:64])
                            nc.scalar.copy(t2_sb[0:1, r0:r0 + 64], t2pp[:])
                            nc.sync.dma_start(
                                out=h2p_sl[r0:r0 + 64, :], in_=hrow2[:]
                            )
                        else:
                            o2 = sb.tile([64, 64], F32, tag="o2")
                            nc.vector.tensor_tensor(
                                out=o2[:], in0=acc[:, 0:64],
                                in1=rec[:].to_broadcast([64, 64]), op=AL.mult,
                            )
                            mx = sb.tile([64, 1], F32, tag="mx")
                            nc.vector.tensor_reduce(
                                out=mx[:], in_=o2[:], axis=AX.X, op=AL.max
                            )
                            mneg = sb.tile([64, 1], F32, tag="mneg")
                            nc.vector.tensor_scalar_mul(mneg[:], mx[:], -1.0)
                            ex = sb.tile([64, 64], F32, tag="ex")
                            nc.scalar.activation(ex[:], o2[:], AF.Exp, bias=mneg[:])
                            sm = sb.tile([64, 1], F32, tag="sm")
                            nc.vector.tensor_reduce(
                                out=sm[:], in_=ex[:], axis=AX.X, op=AL.add
                            )
                            ln = sb.tile([64, 1], F32, tag="ln")
                            nc.scalar.activation(ln[:], sm[:], AF.Ln)
                            mml = sb.tile([64, 1], F32, tag="mml")
                            nc.vector.tensor_tensor(
                                out=mml[:], in0=mx[:], in1=ln[:], op=AL.add
                            )
                            res = sb.tile([64, 64], F32, tag="res")
                            nc.vector.tensor_tensor(
                                out=res[:], in0=o2[:],
                                in1=mml[:].to_broadcast([64, 64]), op=AL.subtract,
                            )
                            nc.sync.dma_start(
                                out=out_fin[r0:r0 + 64, :], in_=res[:]
                            )

            edge_layer(1)
            nc.gpsimd.collective_compute(
                "AllGather",
                AL.bypass,
                replica_groups=[list(range(NCORES))],
                ins=[h2p_sl.ap().opt()],
                outs=[h2p_all.ap().opt()],
            )
            edge_layer(2)

    return nc


def kernel(**inputs):
    from concourse.bass_utils import run_bass_kernel_spmd
    from concourse.library_overlay import lower_extended_insts

    x = np.asarray(inputs["x"], np.float32)
    ei = np.asarray(inputs["edge_index"])
    meta, eidx, REL = prep_structures(ei)
    W2 = np.asarray(inputs["W2"], np.float32)
    b2 = np.asarray(inputs["b2"], np.float32)
    a1w = np.asarray(inputs["a1_w"], np.float32)
    a2w = np.asarray(inputs["a2_w"], np.float32)
    consts = dict(
        W1=np.asarray(inputs["W1"], np.float32),
        b1=np.asarray(inputs["b1"], np.float32),
        W2=W2,
        b2=b2,
        a1h=a1w[:HID_F], a1t=a1w[HID_F:],
        a1b=float(np.asarray(inputs["a1_b"], np.float32)[0]),
        w2a2h=W2 @ a2w[:OUT_F],
        w2a2t=W2 @ a2w[OUT_F:],
        u2bias=float(a2w[:OUT_F] @ b2),
        t2bias=float(a2w[OUT_F:] @ b2 + np.asarray(inputs["a2_b"], np.float32)[0]),
    )
    f16 = os.environ.get("GNN_F32", "0") != "1"
    nc = build_bass(meta, consts, f16=f16)
    _split_multi_waits(nc)
    lower_extended_insts(nc)

    np_ed = np.float16 if f16 else np.float32
    in_maps = []
    for k in range(NCORES):
        xs = np.zeros((ROWS, IN_F), np.float32)
        xs[:NPC] = x[k * NPC:(k + 1) * NPC]
        in_maps.append(
            {
                "x_slT": np.ascontiguousarray(xs.T),
                "eidx": np.ascontiguousarray(eidx[k]),
                "tgtrel": np.ascontiguousarray(REL[k].astype(np_ed)),
            }
        )

    trace = os.environ.get("GNN_TRACE", "0") == "1"
    if trace:
        try:
            import types
            from trn_agent_boot.trn_boot import _ntff_profile_via_ctypes
            _h = _ntff_profile_via_ctypes("/opt/axon/libaxon_pjrt.so")
            m = types.ModuleType("antenv.axon_hooks")
            m.get_axon_ntff_profile_hook = lambda: _h
            sys.modules["antenv.axon_hooks"] = m
        except Exception as e:
            print("profile hook setup failed:", e)
            trace = False
    res = run_bass_kernel_spmd(
        nc, in_maps, core_ids=list(range(NCORES)), trace=trace
    )
    kernel.last_results = res
    out = np.concatenate(
        [res.results[k]["out_fin"][:NPC] for k in range(NCORES)], axis=0
    )
    return out.astype(np.float32)


# revision 33
# speedup vs baseline: 1.0568x; 1.0568x over previous
import os
import sys

sys.path.insert(0, "/opt/trn_rl_repo")

import numpy as np

# ---------------------------------------------------------------- problem dims
NCORES = 8
N = 50000
E = 800000
IN_F, HID_F, OUT_F = 256, 128, 64
NEG = 0.2
EPS = 1e-16

NPC = N // NCORES            # 6250 target nodes per core
BPB = 32                     # targets per block (one-hot width)
NB = (NPC + BPB - 1) // BPB  # 196 blocks per core
GPB = 4                      # blocks per group (PSUM packs 4x32 targets)
NG = NB // GPB               # 49 groups
ROWS = NB * BPB              # 6272 padded rows per core slice
NGA = 25                     # phase-1 groups in table half A
RA = NGA * 128               # 3200 rows per core in half A
RB = ROWS - RA               # 3072 rows per core in half B
TBLA = NCORES * RA           # 25600 rows (< 32768: int16-safe)
TBLB = NCORES * RB           # 24576 rows
WSHIFT = 8.0                 # global exp shift (cancels in normalization)
NW1 = 67                     # L1 aggregated width: v(64) + u2pre + t2pre + 1
NW2 = 65                     # L2 aggregated width: h2(64) + 1


def prep_structures(edge_index):
    """Host-side layout of the edge list.

    Slot storage per group g (4 blocks of 32 targets):
      [half-A slots of b0..b3 | half-B slots of b0..b3]
    so each group needs exactly TWO dma_gather calls (one per table half;
    each half has < 32768 rows so int16 indices cover it directly).
    Slot counts are uniform across cores (compile-time structure); unused
    idx positions hold dummy index 0 with REL=-1 (contributes nothing), so
    num_idxs_reg == num_idxs uniformly across cores.
    Returns meta + per-core eidx [128, TOTCOLS] int16 and REL [128, S_TOT].
    """
    src = edge_index[0].astype(np.int64)
    tgt = edge_index[1].astype(np.int64)
    s_core = src // NPC
    s_r = src % NPC
    in_b = s_r >= RA
    adj = np.where(in_b, s_core * RB + (s_r - RA), s_core * RA + s_r)

    order = np.argsort(tgt, kind="stable")
    src_a = adj[order]
    in_b_s = in_b[order]
    tgt_s = tgt[order]

    core_of = tgt_s // NPC
    tc = tgt_s % NPC
    blk_of = tc // BPB
    rel_of = tc % BPB
    gb = core_of * NB + blk_of
    bounds = np.searchsorted(gb, np.arange(NCORES * NB + 1))

    # per (core, block): A/B (idx, rel) lists sorted by idx
    per_kb = {}
    lo_cnt = np.zeros((NCORES, NB), dtype=np.int64)
    hi_cnt = np.zeros((NCORES, NB), dtype=np.int64)
    for k in range(NCORES):
        for b in range(NB):
            s, e = bounds[k * NB + b], bounds[k * NB + b + 1]
            sa = src_a[s:e]
            rl = rel_of[s:e]
            m = ~in_b_s[s:e]
            lo_o = np.argsort(sa[m], kind="stable")
            hi_o = np.argsort(sa[~m], kind="stable")
            per_kb[(k, b)] = (sa[m][lo_o], rl[m][lo_o], sa[~m][hi_o], rl[~m][hi_o])
            lo_cnt[k, b] = int(m.sum())
            hi_cnt[k, b] = int((~m).sum())

    # uniform-across-cores slot counts per block
    nlo = (lo_cnt.max(axis=0) + 127) // 128    # [NB]
    nhi = (hi_cnt.max(axis=0) + 127) // 128
    NLO_g = np.array([nlo[g * GPB:(g + 1) * GPB].sum() for g in range(NG)])
    NHI_g = np.array([nhi[g * GPB:(g + 1) * GPB].sum() for g in range(NG)])
    S_g = NLO_g + NHI_g
    gs_off = np.concatenate([[0], np.cumsum(S_g)])
    S_TOT = int(gs_off[-1])
    SMAX = int(S_g.max())

    REL = np.full((NCORES, 128, S_TOT), -1.0, dtype=np.float32)

    # segments[g][q] = (lo_s0, nlo_q, hi_s0, nhi_q) slot offsets within group
    segments = []
    for g in range(NG):
        segs = []
        lo_c = 0
        hi_c = int(NLO_g[g])
        for q in range(GPB):
            b = g * GPB + q
            segs.append((lo_c, int(nlo[b]), hi_c, int(nhi[b])))
            lo_c += int(nlo[b])
            hi_c += int(nhi[b])
        segments.append(segs)

    # calls: per group: half-A call then half-B call
    idx_parts = [[] for _ in range(NCORES)]
    calls = []
    col_off = 0
    for g in range(NG):
        for is_b in (False, True):
            n_slots = int((NHI_g if is_b else NLO_g)[g])
            base_s = int(gs_off[g]) + (int(NLO_g[g]) if is_b else 0)
            n_idx = n_slots * 128
            cols = n_idx // 16
            calls.append((col_off, cols, n_idx, base_s, n_slots, is_b, g))
            col_off += cols
            for k in range(NCORES):
                vals = np.zeros(n_idx, dtype=np.int64)
                rr = np.full(n_idx, -1.0, dtype=np.float32)
                pos = 0
                for q in range(GPB):
                    b = g * GPB + q
                    la, lr, ha, hr = per_kb[(k, b)]
                    arr, rl = (ha, hr) if is_b else (la, lr)
                    seg_slots = int((nhi if is_b else nlo)[b])
                    nr = len(arr)
                    vals[pos:pos + nr] = arr
                    rr[pos:pos + nr] = rl
                    pos += seg_slots * 128
                REL[k][:, base_s:base_s + n_slots] = (
                    rr.reshape(n_slots, 128).T
                )
                w16 = vals.reshape(-1, 16).T  # [16, cols]
                idx_parts[k].append(np.tile(w16, (8, 1)).astype(np.int16))
    eidx = [np.concatenate(idx_parts[k], axis=1) for k in range(NCORES)]

    gcalls = [[] for _ in range(NG)]
    for ci, c in enumerate(calls):
        gcalls[c[6]].append(ci)
    gcol = []
    for g in range(NG):
        cs_ = [calls[ci] for ci in gcalls[g]]
        lo_c = min(c[0] for c in cs_)
        hi_c = max(c[0] + c[1] for c in cs_)
        gcol.append((lo_c, hi_c))

    meta = dict(
        S_g=S_g, gs_off=gs_off, S_TOT=S_TOT, SMAX=SMAX,
        calls=calls, gcalls=gcalls, gcol=gcol, TOTCOLS=col_off,
        nlo=nlo, nhi=nhi, segments=segments,
    )
    return meta, eidx, REL


# ------------------------------------------------------------------ host model
def host_model(inputs, f16=True):
    """Numpy mirror of the device dataflow (for algorithm validation)."""
    x = np.asarray(inputs["x"], np.float32)
    ei = np.asarray(inputs["edge_index"])
    W1 = np.asarray(inputs["W1"], np.float32)
    b1 = np.asarray(inputs["b1"], np.float32)
    a1w = np.asarray(inputs["a1_w"], np.float32)
    a1b = np.asarray(inputs["a1_b"], np.float32)
    W2 = np.asarray(inputs["W2"], np.float32)
    b2 = np.asarray(inputs["b2"], np.float32)
    a2w = np.asarray(inputs["a2_w"], np.float32)
    a2b = np.asarray(inputs["a2_b"], np.float32)

    meta, eidx, REL = prep_structures(ei)
    ed = np.float16 if f16 else np.float32
    a1h, a1t = a1w[:HID_F], a1w[HID_F:]
    a2h, a2t = a2w[:OUT_F], a2w[OUT_F:]
    w2a2h = W2 @ a2h
    w2a2t = W2 @ a2t
    u2bias = float(a2h @ b2)
    t2bias = float(a2t @ b2 + a2b[0])

    # ---- phase 1: per-core table1 rows [u1, v(64), u2pre, t2pre, 1, junk]
    t1_sl, tbl_sl = [], []
    for k in range(NCORES):
        xs = np.zeros((ROWS, IN_F), np.float32)
        xs[:NPC] = x[k * NPC:(k + 1) * NPC]
        h = xs @ W1 + b1
        h = np.where(h > 0, h, np.expm1(np.minimum(h, 0.0)))
        row = np.zeros((ROWS, 128), np.float32)
        row[:, 0] = h @ a1h
        row[:, 1:65] = h @ W2
        row[:, 65] = h @ w2a2h
        row[:, 66] = h @ w2a2t
        row[:, 67] = 1.0
        t1_sl.append((h @ a1t + a1b[0]).astype(np.float32))
        tbl_sl.append(row.astype(ed))
    table1a = np.concatenate([t[:RA] for t in tbl_sl], axis=0)
    table1b = np.concatenate([t[RA:] for t in tbl_sl], axis=0)

    def edge_phase(k, tables, t_sl, nw):
        """Returns acc [ROWS, nw] (unnormalized sums + denominator)."""
        acc = np.zeros((ROWS, nw), np.float32)
        relk = REL[k]
        gs_off = meta["gs_off"]
        for g in range(NG):
            S = int(meta["S_g"][g])
            so = int(gs_off[g])
            # gather (all idx positions valid; dummies point at row 0)
            gt = np.zeros((128, S, 128), ed)
            for ci in meta["gcalls"][g]:
                c0, cols, n_idx, base_s, n_slots, is_b, _g = meta["calls"][ci]
                if n_slots == 0:
                    continue
                w16 = eidx[k][:16, c0:c0 + cols]
                flat = w16.T.flatten()[:n_idx].astype(np.int64)
                rows = tables[1 if is_b else 0][flat].astype(ed)
                gt[:, base_s - so:base_s - so + n_slots, :] = np.transpose(
                    rows.reshape(n_slots, 128, 128), (1, 0, 2)
                )
            trow = np.zeros((NG * 128,), np.float32)
            trow[:ROWS] = t_sl
            trowg = trow[g * 128:(g + 1) * 128].astype(ed)  # [128]
            rel_g = relk[:, so:so + S]  # [128, S]
            iota = np.arange(BPB, dtype=np.float32)
            Mx = (rel_g[:, :, None] == iota[None, None, :]).astype(ed)  # [128,S,32]
            # tsel: per (q, segment)
            tsel = np.zeros((128, S, BPB), ed)
            for q in range(GPB):
                tw = trowg[q * BPB:(q + 1) * BPB]
                lo_s0, nlo_q, hi_s0, nhi_q = meta["segments"][g][q]
                for (s0, ns_) in ((lo_s0, nlo_q), (hi_s0, nhi_q)):
                    tsel[:, s0:s0 + ns_, :] = (
                        Mx[:, s0:s0 + ns_, :] * tw[None, None, :]
                    )
            ted = tsel.sum(axis=2, dtype=np.float32).astype(ed)  # [128, S]
            z = (gt[:, :, 0].astype(np.float32) + ted.astype(np.float32))
            zl = np.maximum(z, NEG * z)
            w = np.exp(zl - WSHIFT).astype(ed)
            Wm = (Mx * w[:, :, None]).astype(ed)  # [128, S, 32]
            for q in range(GPB):
                r0 = g * 128 + q * BPB
                a = np.zeros((BPB, nw), np.float32)
                lo_s0, nlo_q, hi_s0, nhi_q = meta["segments"][g][q]
                slots = list(range(lo_s0, lo_s0 + nlo_q)) + list(
                    range(hi_s0, hi_s0 + nhi_q)
                )
                for s in slots:
                    a += (
                        Wm[:, s, :].astype(np.float32).T
                        @ gt[:, s, 1:1 + nw].astype(np.float32)
                    )
                acc[r0:r0 + BPB] = a
        return acc

    # ---- L1 edge phase + L2 table build
    t2_sl, tbl2_sl = [], []
    for k in range(NCORES):
        acc = edge_phase(k, (table1a, table1b), t1_sl[k], NW1)
        den = acc[:, 66:67] + EPS
        h2 = acc[:, 0:64] / den + b2
        u2 = acc[:, 64] / den[:, 0] + u2bias
        t2 = acc[:, 65] / den[:, 0] + t2bias
        row = np.zeros((ROWS, 128), np.float32)
        row[:, 0] = u2
        row[:, 1:65] = h2
        row[:, 65] = 1.0
        t2_sl.append(t2.astype(np.float32))
        tbl2_sl.append(row.astype(ed))
    table2a = np.concatenate([t[:RA] for t in tbl2_sl], axis=0)
    table2b = np.concatenate([t[RA:] for t in tbl2_sl], axis=0)
    host_model.t1_sl = t1_sl
    host_model.t2_sl = t2_sl

    outs = []
    for k in range(NCORES):
        acc = edge_phase(k, (table2a, table2b), t2_sl[k], NW2)
        den = acc[:, 64:65] + EPS
        o2 = acc[:, 0:64] / den
        m = o2.max(axis=1, keepdims=True)
        lse = np.log(np.exp(o2 - m).sum(axis=1, keepdims=True)) + m
        outs.append((o2 - lse)[:NPC])
    return np.concatenate(outs, axis=0).astype(np.float32)


if __name__ == "__main__":
    sys.path.insert(0, os.path.dirname(os.path.abspath(__file__)))
    import reference

    inputs = {k: np.asarray(v) for k, v in reference.setup_inputs().items()}
    expect = np.asarray(reference.reference(**inputs))
    for f16 in (True, False):
        got = host_model(inputs, f16=f16)
        err = np.abs(got - expect)
        rel = err.max() / np.abs(expect).max()
        print(f"host_model f16={f16}: absmax {err.max():.3e} rel {rel:.3e}")


# ------------------------------------------------------------------ bass build
def _patch_tile_drain():
    """This walrus build supports only one sync-wait per SP TPB_CTRL
    instruction; split TileContext's exit drain into single-wait NOPs."""
    import concourse.mybir as mybir
    import concourse.tile as tile
    from concourse.tile import ScopedClock

    if getattr(tile.TileContext, "_drain_split_patched", False):
        return

    def _split(self, tick_clock, wait_clock):
        nop0 = self.nc.sync.nop()
        wait_clock.add_sem_waits(
            nop0.ins, ScopedClock({None: tick_clock.global_clock})
        )
        si = nop0.ins.sync_info
        if si is not None and si.on_wait and len(si.on_wait) > 1:
            waits = list(si.on_wait)
            nop0.ins.sync_info = mybir.SyncInfo(
                on_wait=[waits[0]], on_update=list(si.on_update)
            )
            for w in waits[1:]:
                n = self.nc.sync.nop()
                n.ins.sync_info = mybir.SyncInfo(on_wait=[w], on_update=[])
        self.nc.sync.drain()
        self.nc.all_engine_barrier()
        popped = self.nc._tile_sem_poison_stack.pop()
        assert popped is self._sem_poison
        self.nc.clear_and_free_semaphores(list(self.sems.allocated().values()))
        self.nc.all_engine_barrier()

    tile.TileContext._drain_and_barrier = _split
    tile.TileContext._drain_split_patched = True


def _split_multi_waits(nc):
    """Move extra sync waits onto single-wait NOPs (walrus build limit)."""
    import concourse.mybir as mybir

    ctr = [0]
    for f in nc.m.functions:
        for bb in f.blocks:
            insts = list(bb.instructions)
            out = []
            changed = False
            for ins in insts:
                si = getattr(ins, "sync_info", None)
                if si is not None and si.on_wait and len(si.on_wait) > 1:
                    waits = list(si.on_wait)
                    for w in waits[:-1]:
                        n = mybir.InstNoOp(
                            name=f"splitw-{ctr[0]}", ins=[], outs=[]
                        )
                        ctr[0] += 1
                        n.engine = ins.engine
                        n.sync_info = mybir.SyncInfo(on_wait=[w], on_update=[])
                        nc.register_instruction(n)
                        out.append(n)
                    ins.sync_info = mybir.SyncInfo(
                        on_wait=[waits[-1]], on_update=list(si.on_update)
                    )
                    changed = True
                out.append(ins)
            if changed:
                bb.instructions = out


def build_bass(meta, consts, f16=True):
    import concourse.bass as bass
    import concourse.mybir as mybir
    import concourse.tile as tile
    from concourse.library_config import mlp as mlp_lib
    from concourse.tile_rust import add_dep_helper

    _patch_tile_drain()

    F32 = mybir.dt.float32
    F16 = mybir.dt.float16 if f16 else mybir.dt.float32
    I16 = mybir.dt.int16
    AL = mybir.AluOpType
    AF = mybir.ActivationFunctionType
    AX = mybir.AxisListType

    S_TOT = meta["S_TOT"]
    SMAX = meta["SMAX"]
    gs_off = meta["gs_off"]
    segments = meta["segments"]
    TOTCOLS = meta["TOTCOLS"]

    nc = bass.Bass(
        num_devices=NCORES, num_swdge_queues=4,
        dynamic_dma_scratch_size=32768,
    )

    x_slT = nc.dram_tensor("x_slT", [IN_F, ROWS], F32, kind="ExternalInput")
    eidx = nc.dram_tensor("eidx", [128, meta["TOTCOLS"]], I16, kind="ExternalInput")
    tgtrel = nc.dram_tensor("tgtrel", [128, S_TOT], F16, kind="ExternalInput")
    out_fin = nc.dram_tensor("out_fin", [ROWS, OUT_F], F32, kind="ExternalOutput")

    def inl(name, arr):
        return nc.inline_tensor(np.ascontiguousarray(arr), name=name)

    np_ed = np.float16 if f16 else np.float32
    c_W1a = inl("W1a", consts["W1"][:128].astype(np.float32))
    c_W1b = inl("W1b", consts["W1"][128:].astype(np.float32))
    c_W2 = inl("W2c", consts["W2"].astype(np.float32))
    c_a1h = inl("a1h", consts["a1h"].reshape(HID_F, 1).astype(np.float32))
    c_a1t = inl("a1t", consts["a1t"].reshape(HID_F, 1).astype(np.float32))
    c_w2a2h = inl("w2a2h", consts["w2a2h"].reshape(HID_F, 1).astype(np.float32))
    c_w2a2t = inl("w2a2t", consts["w2a2t"].reshape(HID_F, 1).astype(np.float32))
    c_b1 = inl("b1c", consts["b1"].reshape(HID_F, 1).astype(np.float32))
    c_b1n = inl("b1n", (-consts["b1"]).reshape(HID_F, 1).astype(np.float32))
    c_b2bc = inl("b2bc", np.tile(consts["b2"].astype(np.float32), (128, 1)))
    c_iota = inl("iota32", np.tile(np.arange(BPB, dtype=np_ed), (128, 1)))
    c_ones1 = inl("ones1", np.ones((1, 128), np.float32))
    c_I128 = inl("I128", np.eye(128, dtype=np.float32))
    c_b1a = inl("b1a", np.full((1, 1), consts["a1b"], np.float32))
    c_nsh = inl("nsh", np.full((128, 1), -WSHIFT, np.float32))
    c_u2b = inl("u2b", np.full((128, 1), consts["u2bias"], np.float32))
    c_t2b = inl("t2b", np.full((128, 1), consts["t2bias"], np.float32))

    h1a_sl = nc.dram_tensor("h1a_sl", [RA, 128], F16)
    h1b_sl = nc.dram_tensor("h1b_sl", [RB, 128], F16)
    h1a_all = nc.dram_tensor("h1a_all", [TBLA, 128], F16, addr_space="Shared")
    h1b_all = nc.dram_tensor("h1b_all", [TBLB, 128], F16, addr_space="Shared")
    h2a_sl = nc.dram_tensor("h2a_sl", [RA, 128], F16)
    h2b_sl = nc.dram_tensor("h2b_sl", [RB, 128], F16)
    h2a_all = nc.dram_tensor("h2a_all", [TBLA, 128], F16, addr_space="Shared")
    h2b_all = nc.dram_tensor("h2b_all", [TBLB, 128], F16, addr_space="Shared")

    def allgather(src, dst):
        import concourse.mybir as _mb
        nc.gpsimd.collective_compute(
            "AllGather",
            _mb.AluOpType.bypass,
            replica_groups=[list(range(NCORES))],
            ins=[src.ap().opt()],
            outs=[dst.ap().opt()],
        )

    with tile.TileContext(nc) as tc:
        import contextlib

        with contextlib.ExitStack() as ctx:
            cpool = ctx.enter_context(tc.tile_pool(name="consts", bufs=1))
            persist = ctx.enter_context(tc.tile_pool(name="persist", bufs=1))
            sb = ctx.enter_context(tc.tile_pool(name="sb", bufs=4))
            gp = ctx.enter_context(tc.tile_pool(name="gp", bufs=4))
            ps = ctx.enter_context(tc.tile_pool(name="ps", bufs=4, space="PSUM"))
            psa = ctx.enter_context(tc.tile_pool(name="psa", bufs=2, space="PSUM"))

            def cload(handle, shape, dtype):
                t = cpool.tile(shape, dtype, tag=handle.name)
                nc.sync.dma_start(out=t[:], in_=handle[:, :])
                return t

            W1a = cload(c_W1a, [128, 128], F32)
            W1b = cload(c_W1b, [128, 128], F32)
            W2 = cload(c_W2, [128, 64], F32)
            a1h = cload(c_a1h, [128, 1], F32)
            a1t = cload(c_a1t, [128, 1], F32)
            w2a2h = cload(c_w2a2h, [128, 1], F32)
            w2a2t = cload(c_w2a2t, [128, 1], F32)
            b1c = cload(c_b1, [128, 1], F32)
            b1n = cload(c_b1n, [128, 1], F32)
            b2bc = cload(c_b2bc, [128, 64], F32)
            iota = cload(c_iota, [128, BPB], F16)
            ones1 = cload(c_ones1, [1, 128], F32)
            I128 = cload(c_I128, [128, 128], F32)
            b1a = cload(c_b1a, [1, 1], F32)
            nsh = cload(c_nsh, [128, 1], F32)
            u2b = cload(c_u2b, [128, 1], F32)
            t2b = cload(c_t2b, [128, 1], F32)

            t1_sb = persist.tile([1, ROWS], F32, tag="t1")
            t2_sb = persist.tile([1, ROWS], F32, tag="t2")
            trowall = persist.tile([128, NG * 128], F16, tag="trowall")
            trb = persist.tile([128, S_TOT], F16, tag="trb")
            nc.sync.dma_start(out=trb[:], in_=tgtrel[:, :])
            eib = persist.tile([128, TOTCOLS], I16, tag="eib")
            nc.sync.dma_start(out=eib[:], in_=eidx[:, :])

            ll = nc.gpsimd.load_library(mlp_lib)
            nidx_regs = {}
            for c in meta["calls"]:
                ni = c[2]
                if ni not in nidx_regs and ni > 0:
                    r = nc.gpsimd.alloc_register(f"nidx_{ni}")
                    nc.gpsimd.reg_mov(r, ni)
                    nidx_regs[ni] = r

            # warm up gather tiles so pad columns never hold NaN bit patterns
            for _ in range(4):
                gt = gp.tile([128, SMAX, 128], F16, tag="g")
                nc.vector.memset(gt[:], 0.0)

            # ---------------- phase 1: table1 rows + t1 ----------------
            for cix in range(NG):
                r0 = cix * 128
                xT0 = sb.tile([128, 128], F32, tag="xT0")
                xT1 = sb.tile([128, 128], F32, tag="xT1")
                nc.sync.dma_start(out=xT0[:], in_=x_slT[0:128, r0:r0 + 128])
                nc.sync.dma_start(out=xT1[:], in_=x_slT[128:256, r0:r0 + 128])
                hTp = ps.tile([128, 128], F32, tag="pp")
                nc.tensor.matmul(hTp[:], lhsT=W1a[:], rhs=xT0[:], start=True, stop=False)
                nc.tensor.matmul(hTp[:], lhsT=W1b[:], rhs=xT1[:], start=False, stop=True)
                ha = sb.tile([128, 128], F32, tag="ha")
                nc.scalar.activation(ha[:], hTp[:], AF.Relu, bias=b1c[:])
                hcn = sb.tile([128, 128], F32, tag="hcn")
                nc.scalar.activation(hcn[:], hTp[:], AF.Relu, bias=b1n[:], scale=-1.0)
                hdx = sb.tile([128, 128], F32, tag="hdx")
                nc.scalar.activation(hdx[:], hcn[:], AF.Exp, scale=-1.0)
                h1T = sb.tile([128, 128], F32, tag="h1T")
                nc.vector.scalar_tensor_tensor(
                    out=h1T[:], in0=hdx[:], scalar=-1.0, in1=ha[:],
                    op0=AL.add, op1=AL.add,
                )
                P = ps.tile([128, NW1], F32, tag="pp")
                nc.tensor.matmul(P[:, 1:65], lhsT=h1T[:], rhs=W2[:], start=True, stop=True)
                nc.tensor.matmul(P[:, 0:1], lhsT=h1T[:], rhs=a1h[:], start=True, stop=True)
                nc.tensor.matmul(P[:, 65:66], lhsT=h1T[:], rhs=w2a2h[:], start=True, stop=True)
                nc.tensor.matmul(P[:, 66:67], lhsT=h1T[:], rhs=w2a2t[:], start=True, stop=True)
                t1p = ps.tile([1, 128], F32, tag="pp")
                nc.tensor.matmul(t1p[:], lhsT=a1t[:], rhs=h1T[:], start=True, stop=True)
                nc.scalar.activation(
                    t1_sb[0:1, r0:r0 + 128], t1p[:], AF.Identity, bias=b1a[:]
                )
                hrow = sb.tile([128, 128], F16, tag="hrow")
                nc.scalar.copy(hrow[:, 0:NW1], P[:])
                nc.vector.memset(hrow[:, NW1:NW1 + 1], 1.0)
                if cix < NGA:
                    nc.sync.dma_start(out=h1a_sl[r0:r0 + 128, :], in_=hrow[:])
                else:
                    nc.sync.dma_start(
                        out=h1b_sl[r0 - RA:r0 - RA + 128, :], in_=hrow[:]
                    )
                if cix == NGA - 1:
                    allgather(h1a_sl, h1a_all)

            allgather(h1b_sl, h1b_all)

            qctr = [0]

            def edge_layer(layer):
                if layer == 1:
                    tables, t_sb, nw = (h1a_all, h1b_all), t1_sb, NW1
                else:
                    tables, t_sb, nw = (h2a_all, h2b_all), t2_sb, NW2

                # trowall: broadcast t values down partitions, fp16
                for g in range(NG):
                    r0 = g * 128
                    trp = ps.tile([128, 128], F32, tag="pp")
                    nc.tensor.matmul(
                        trp[:], lhsT=ones1[:], rhs=t_sb[0:1, r0:r0 + 128],
                        start=True, stop=True,
                    )
                    nc.scalar.copy(trowall[:, r0:r0 + 128], trp[:])

                for g in range(NG):
                    S = int(meta["S_g"][g])
                    so = int(gs_off[g])
                    gt = gp.tile([128, SMAX, 128], F16, tag="g")
                    for ci in meta["gcalls"][g]:
                        (co, cols, n_idx, base_s, n_slots, is_b, _g) = \
                            meta["calls"][ci]
                        if n_slots == 0:
                            continue
                        tbl_ap = tables[1 if is_b else 0][:, :]
                        qctr[0] += 1
                        gi = nc.gpsimd.dma_gather(
                            gt[:, base_s - so:base_s - so + n_slots, :],
                            tbl_ap,
                            eib[:, co:co + cols],
                            num_idxs=n_idx,
                            num_idxs_reg=nidx_regs[n_idx],
                            elem_size=128,
                            single_packet=False,
                            queue_num=qctr[0] % 4,
                        )
                        add_dep_helper(gi.ins, ll.ins)

                    M = sb.tile([128, SMAX, BPB], F16, tag="M")
                    nc.vector.tensor_tensor(
                        out=M[:, 0:S, :],
                        in0=trb[:, so:so + S].to_broadcast([128, S, BPB]),
                        in1=bass.AP(
                            iota[:].tensor, iota[:].offset,
                            [list(iota[:].ap[0]), [0, S], list(iota[:].ap[1])],
                        ),
                        op=AL.is_equal,
                    )
                    tsel = sb.tile([128, SMAX, BPB], F16, tag="tsel")
                    trg = trowall[:, g * 128:(g + 1) * 128]
                    for q in range(GPB):
                        tq = trg[:, q * BPB:(q + 1) * BPB]
                        lo_s0, nlo_q, hi_s0, nhi_q = segments[g][q]
                        for (s0_, ns_) in ((lo_s0, nlo_q), (hi_s0, nhi_q)):
                            if ns_ == 0:
                                continue
                            nc.vector.tensor_tensor(
                                out=tsel[:, s0_:s0_ + ns_, :],
                                in0=M[:, s0_:s0_ + ns_, :],
                                in1=bass.AP(
                                    tq.tensor, tq.offset,
                                    [list(tq.ap[0]), [0, ns_], list(tq.ap[1])],
                                ),
                                op=AL.mult,
                            )
                    ted = sb.tile([128, SMAX], F16, tag="ted")
                    with nc.allow_low_precision(
                        reason="one nonzero per segment; fp16 exact"
                    ):
                        nc.vector.tensor_reduce(
                            out=ted[:, 0:S], in_=tsel[:, 0:S, :],
                            axis=AX.X, op=AL.add,
                        )
                    # z = g0 + ted ; zl = max(z, 0.2 z) ; w = exp(zl - 8)
                    g0 = bass.AP(
                        gt[:].tensor, gt[:].offset,
                        [list(gt[:].ap[0]), [128, S]],
                    )
                    z = sb.tile([128, SMAX], F16, tag="z")
                    nc.vector.tensor_tensor(
                        out=z[:, 0:S], in0=g0, in1=ted[:, 0:S], op=AL.add
                    )
                    zl = sb.tile([128, SMAX], F16, tag="zl")
                    nc.vector.scalar_tensor_tensor(
                        out=zl[:, 0:S], in0=z[:, 0:S], scalar=NEG,
                        in1=z[:, 0:S], op0=AL.mult, op1=AL.max,
                    )
                    w = sb.tile([128, SMAX], F16, tag="w")
                    nc.scalar.activation(w[:, 0:S], zl[:, 0:S], AF.Exp, bias=nsh[:])
                    Wm = sb.tile([128, SMAX, BPB], F16, tag="Wm")
                    nc.vector.tensor_tensor(
                        out=Wm[:, 0:S, :],
                        in0=M[:, 0:S, :],
                        in1=w[:, 0:S].to_broadcast([128, S, BPB]),
                        op=AL.mult,
                    )

                    acc01 = psa.tile([64, nw], F32, tag="acc01")
                    acc23 = psa.tile([64, nw], F32, tag="acc23")
                    accs = [acc01, acc23]
                    for q in range(GPB):
                        acc = accs[q // 2]
                        qq = q % 2
                        lo_s0, nlo_q, hi_s0, nhi_q = segments[g][q]
                        slots = list(range(lo_s0, lo_s0 + nlo_q)) + list(
                            range(hi_s0, hi_s0 + nhi_q)
                        )
                        for si, sl in enumerate(slots):
                            nc.tensor.matmul(
                                acc[qq * BPB:(qq + 1) * BPB, :],
                                lhsT=Wm[:, sl, :],
                                rhs=gt[:, sl, 1:1 + nw],
                                start=(si == 0), stop=(si == len(slots) - 1),
                                skip_group_check=True,
                            )

                    for half, acc in enumerate(accs):
                        r0 = g * 128 + half * 64
                        den = sb.tile([64, 1], F32, tag="den")
                        nc.vector.tensor_scalar_add(den[:], acc[:, nw - 1:nw], EPS)
                        rec = sb.tile([64, 1], F32, tag="rec")
                        nc.vector.reciprocal(rec[:], den[:])
                        if layer == 1:
                            h2p = sb.tile([64, 64], F32, tag="h2p")
                            nc.vector.tensor_tensor(
                                out=h2p[:], in0=acc[:, 0:64],
                                in1=rec[:].to_broadcast([64, 64]), op=AL.mult,
                            )
                            h2 = sb.tile([64, 64], F32, tag="h2")
                            nc.vector.tensor_tensor(
                                out=h2[:], in0=h2p[:], in1=b2bc[0:64, :], op=AL.add
                            )
                            hrow2 = sb.tile([64, 128], F16, tag="hrow2")
                            nc.scalar.activation(
                                hrow2[:, 0:1], acc[:, 64:65], AF.Identity,
                                scale=rec[:], bias=u2b[0:64, :],
                            )
                            nc.scalar.copy(hrow2[:, 1:65], h2[:])
                            nc.vector.memset(hrow2[:, 65:66], 1.0)
                            t2c = sb.tile([64, 1], F32, tag="t2c")
                            nc.scalar.activation(
                                t2c[:], acc[:, 65:66], AF.Identity,
                                scale=rec[:], bias=t2b[0:64, :],
                            )
                            t2pp = ps.tile([1, 64], F32, tag="pp")
                            nc.tensor.transpose(t2pp[:], t2c[:], I128[0:64, 0:64])
                            nc.scalar.copy(t2_sb[0:1, r0:r0 + 64], t2pp[:])
                            if r0 < RA:
                                nc.sync.dma_start(
                                    out=h2a_sl[r0:r0 + 64, :], in_=hrow2[:]
                                )
                            else:
                                nc.sync.dma_start(
                                    out=h2b_sl[r0 - RA:r0 - RA + 64, :],
                                    in_=hrow2[:],
                                )
                        else:
                            o2 = sb.tile([64, 64], F32, tag="o2")
                            nc.vector.tensor_tensor(
                                out=o2[:], in0=acc[:, 0:64],
                                in1=rec[:].to_broadcast([64, 64]), op=AL.mult,
                            )
                            mx = sb.tile([64, 1], F32, tag="mx")
                            nc.vector.tensor_reduce(
                                out=mx[:], in_=o2[:], axis=AX.X, op=AL.max
                            )
                            mneg = sb.tile([64, 1], F32, tag="mneg")
                            nc.vector.tensor_scalar_mul(mneg[:], mx[:], -1.0)
                            ex = sb.tile([64, 64], F32, tag="ex")
                            nc.scalar.activation(ex[:], o2[:], AF.Exp, bias=mneg[:])
                            sm = sb.tile([64, 1], F32, tag="sm")
                            nc.vector.tensor_reduce(
                                out=sm[:], in_=ex[:], axis=AX.X, op=AL.add
                            )
                            ln = sb.tile([64, 1], F32, tag="ln")
                            nc.scalar.activation(ln[:], sm[:], AF.Ln)
                            mml = sb.tile([64, 1], F32, tag="mml")
                            nc.vector.tensor_tensor(
                                out=mml[:], in0=mx[:], in1=ln[:], op=AL.add
                            )
                            res = sb.tile([64, 64], F32, tag="res")
                            nc.vector.tensor_tensor(
                                out=res[:], in0=o2[:],
                                in1=mml[:].to_broadcast([64, 64]), op=AL.subtract,
                            )
                            nc.sync.dma_start(
                                out=out_fin[r0:r0 + 64, :], in_=res[:]
                            )
                    if layer == 1 and g == NGA - 1:
                        allgather(h2a_sl, h2a_all)

            edge_layer(1)
            allgather(h2b_sl, h2b_all)
            edge_layer(2)

    return nc


def kernel(**inputs):
    from concourse.bass_utils import run_bass_kernel_spmd
    from concourse.library_overlay import lower_extended_insts

    x = np.asarray(inputs["x"], np.float32)
    ei = np.asarray(inputs["edge_index"])
    meta, eidx, REL = prep_structures(ei)
    W2 = np.asarray(inputs["W2"], np.float32)
    b2 = np.asarray(inputs["b2"], np.float32)
    a1w = np.asarray(inputs["a1_w"], np.float32)
    a2w = np.asarray(inputs["a2_w"], np.float32)
    consts = dict(
        W1=np.asarray(inputs["W1"], np.float32),
        b1=np.asarray(inputs["b1"], np.float32),
        W2=W2,
        b2=b2,
        a1h=a1w[:HID_F], a1t=a1w[HID_F:],
        a1b=float(np.asarray(inputs["a1_b"], np.float32)[0]),
        w2a2h=W2 @ a2w[:OUT_F],
        w2a2t=W2 @ a2w[OUT_F:],
        u2bias=float(a2w[:OUT_F] @ b2),
        t2bias=float(a2w[OUT_F:] @ b2 + np.asarray(inputs["a2_b"], np.float32)[0]),
    )
    f16 = os.environ.get("GNN_F32", "0") != "1"
    nc = build_bass(meta, consts, f16=f16)
    _split_multi_waits(nc)
    lower_extended_insts(nc)

    np_ed = np.float16 if f16 else np.float32
    in_maps = []
    for k in range(NCORES):
        xs = np.zeros((ROWS, IN_F), np.float32)
        xs[:NPC] = x[k * NPC:(k + 1) * NPC]
        in_maps.append(
            {
                "x_slT": np.ascontiguousarray(xs.T),
                "eidx": np.ascontiguousarray(eidx[k]),
                "tgtrel": np.ascontiguousarray(REL[k].astype(np_ed)),
            }
        )

    trace = os.environ.get("GNN_TRACE", "0") == "1"
    if trace:
        try:
            import types
            from trn_agent_boot.trn_boot import _ntff_profile_via_ctypes
            _h = _ntff_profile_via_ctypes("/opt/axon/libaxon_pjrt.so")
            m = types.ModuleType("antenv.axon_hooks")
            m.get_axon_ntff_profile_hook = lambda: _h
            sys.modules["antenv.axon_hooks"] = m
        except Exception as e:
            print("profile hook setup failed:", e)
            trace = False
    res = run_bass_kernel_spmd(
        nc, in_maps, core_ids=list(range(NCORES)), trace=trace
    )
    kernel.last_results = res
    out = np.concatenate(
        [res.results[k]["out_fin"][:NPC] for k in range(NCORES)], axis=0
    )
    return out.astype(np.float32)


# revision 35
# speedup vs baseline: 1.0636x; 1.0065x over previous
import os
import sys

sys.path.insert(0, "/opt/trn_rl_repo")

import numpy as np

# ---------------------------------------------------------------- problem dims
NCORES = 8
N = 50000
E = 800000
IN_F, HID_F, OUT_F = 256, 128, 64
NEG = 0.2
EPS = 1e-16

NPC = N // NCORES            # 6250 target nodes per core
BPB = 32                     # targets per block (one-hot width)
NB = (NPC + BPB - 1) // BPB  # 196 blocks per core
GPB = 4                      # blocks per group (PSUM packs 4x32 targets)
NG = NB // GPB               # 49 groups
ROWS = NB * BPB              # 6272 padded rows per core slice
NGA = 25                     # phase-1 groups in table half A
RA = NGA * 128               # 3200 rows per core in half A
RB = ROWS - RA               # 3072 rows per core in half B
TBLA = NCORES * RA           # 25600 rows (< 32768: int16-safe)
TBLB = NCORES * RB           # 24576 rows
WSHIFT = 8.0                 # global exp shift (cancels in normalization)
NW1 = 67                     # L1 aggregated width: v(64) + u2pre + t2pre + 1
NW2 = 65                     # L2 aggregated width: h2(64) + 1


def prep_structures(edge_index):
    """Host-side layout of the edge list.

    Slot storage per group g (4 blocks of 32 targets):
      [half-A slots of b0..b3 | half-B slots of b0..b3]
    so each group needs exactly TWO dma_gather calls (one per table half;
    each half has < 32768 rows so int16 indices cover it directly).
    Slot counts are uniform across cores (compile-time structure); unused
    idx positions hold dummy index 0 with REL=-1 (contributes nothing), so
    num_idxs_reg == num_idxs uniformly across cores.
    Returns meta + per-core eidx [128, TOTCOLS] int16 and REL [128, S_TOT].
    """
    src = edge_index[0].astype(np.int64)
    tgt = edge_index[1].astype(np.int64)
    s_core = src // NPC
    s_r = src % NPC
    in_b = s_r >= RA
    adj = np.where(in_b, s_core * RB + (s_r - RA), s_core * RA + s_r)

    order = np.argsort(tgt, kind="stable")
    src_a = adj[order]
    in_b_s = in_b[order]
    tgt_s = tgt[order]

    core_of = tgt_s // NPC
    tc = tgt_s % NPC
    blk_of = tc // BPB
    rel_of = tc % BPB
    gb = core_of * NB + blk_of
    bounds = np.searchsorted(gb, np.arange(NCORES * NB + 1))

    # per (core, block): A/B (idx, rel) lists sorted by idx
    per_kb = {}
    lo_cnt = np.zeros((NCORES, NB), dtype=np.int64)
    hi_cnt = np.zeros((NCORES, NB), dtype=np.int64)
    for k in range(NCORES):
        for b in range(NB):
            s, e = bounds[k * NB + b], bounds[k * NB + b + 1]
            sa = src_a[s:e]
            rl = rel_of[s:e]
            m = ~in_b_s[s:e]
            lo_o = np.argsort(sa[m], kind="stable")
            hi_o = np.argsort(sa[~m], kind="stable")
            per_kb[(k, b)] = (sa[m][lo_o], rl[m][lo_o], sa[~m][hi_o], rl[~m][hi_o])
            lo_cnt[k, b] = int(m.sum())
            hi_cnt[k, b] = int((~m).sum())

    # uniform-across-cores slot counts per block
    nlo = (lo_cnt.max(axis=0) + 127) // 128    # [NB]
    nhi = (hi_cnt.max(axis=0) + 127) // 128
    NLO_g = np.array([nlo[g * GPB:(g + 1) * GPB].sum() for g in range(NG)])
    NHI_g = np.array([nhi[g * GPB:(g + 1) * GPB].sum() for g in range(NG)])
    S_g = NLO_g + NHI_g
    gs_off = np.concatenate([[0], np.cumsum(S_g)])
    S_TOT = int(gs_off[-1])
    SMAX = int(S_g.max())

    REL = np.full((NCORES, 128, S_TOT), -1.0, dtype=np.float32)

    # segments[g][q] = (lo_s0, nlo_q, hi_s0, nhi_q) slot offsets within group
    segments = []
    for g in range(NG):
        segs = []
        lo_c = 0
        hi_c = int(NLO_g[g])
        for q in range(GPB):
            b = g * GPB + q
            segs.append((lo_c, int(nlo[b]), hi_c, int(nhi[b])))
            lo_c += int(nlo[b])
            hi_c += int(nhi[b])
        segments.append(segs)

    # calls: per group: half-A call then half-B call
    idx_parts = [[] for _ in range(NCORES)]
    calls = []
    col_off = 0
    for g in range(NG):
        for is_b in (False, True):
            n_slots = int((NHI_g if is_b else NLO_g)[g])
            base_s = int(gs_off[g]) + (int(NLO_g[g]) if is_b else 0)
            n_idx = n_slots * 128
            cols = n_idx // 16
            calls.append((col_off, cols, n_idx, base_s, n_slots, is_b, g))
            col_off += cols
            for k in range(NCORES):
                vals = np.zeros(n_idx, dtype=np.int64)
                rr = np.full(n_idx, -1.0, dtype=np.float32)
                pos = 0
                for q in range(GPB):
                    b = g * GPB + q
                    la, lr, ha, hr = per_kb[(k, b)]
                    arr, rl = (ha, hr) if is_b else (la, lr)
                    seg_slots = int((nhi if is_b else nlo)[b])
                    nr = len(arr)
                    vals[pos:pos + nr] = arr
                    rr[pos:pos + nr] = rl
                    pos += seg_slots * 128
                REL[k][:, base_s:base_s + n_slots] = (
                    rr.reshape(n_slots, 128).T
                )
                w16 = vals.reshape(-1, 16).T  # [16, cols]
                idx_parts[k].append(np.tile(w16, (8, 1)).astype(np.int16))
    eidx = [np.concatenate(idx_parts[k], axis=1) for k in range(NCORES)]

    gcalls = [[] for _ in range(NG)]
    for ci, c in enumerate(calls):
        gcalls[c[6]].append(ci)
    gcol = []
    for g in range(NG):
        cs_ = [calls[ci] for ci in gcalls[g]]
        lo_c = min(c[0] for c in cs_)
        hi_c = max(c[0] + c[1] for c in cs_)
        gcol.append((lo_c, hi_c))

    meta = dict(
        S_g=S_g, gs_off=gs_off, S_TOT=S_TOT, SMAX=SMAX,
        calls=calls, gcalls=gcalls, gcol=gcol, TOTCOLS=col_off,
        nlo=nlo, nhi=nhi, segments=segments,
    )
    return meta, eidx, REL


# ------------------------------------------------------------------ host model
def host_model(inputs, f16=True):
    """Numpy mirror of the device dataflow (for algorithm validation)."""
    x = np.asarray(inputs["x"], np.float32)
    ei = np.asarray(inputs["edge_index"])
    W1 = np.asarray(inputs["W1"], np.float32)
    b1 = np.asarray(inputs["b1"], np.float32)
    a1w = np.asarray(inputs["a1_w"], np.float32)
    a1b = np.asarray(inputs["a1_b"], np.float32)
    W2 = np.asarray(inputs["W2"], np.float32)
    b2 = np.asarray(inputs["b2"], np.float32)
    a2w = np.asarray(inputs["a2_w"], np.float32)
    a2b = np.asarray(inputs["a2_b"], np.float32)

    meta, eidx, REL = prep_structures(ei)
    ed = np.float16 if f16 else np.float32
    a1h, a1t = a1w[:HID_F], a1w[HID_F:]
    a2h, a2t = a2w[:OUT_F], a2w[OUT_F:]
    w2a2h = W2 @ a2h
    w2a2t = W2 @ a2t
    u2bias = float(a2h @ b2)
    t2bias = float(a2t @ b2 + a2b[0])

    # ---- phase 1: per-core table1 rows [u1, v(64), u2pre, t2pre, 1, junk]
    t1_sl, tbl_sl = [], []
    for k in range(NCORES):
        xs = np.zeros((ROWS, IN_F), np.float32)
        xs[:NPC] = x[k * NPC:(k + 1) * NPC]
        h = xs @ W1 + b1
        h = np.where(h > 0, h, np.expm1(np.minimum(h, 0.0)))
        row = np.zeros((ROWS, 128), np.float32)
        row[:, 0] = h @ a1h
        row[:, 1:65] = h @ W2
        row[:, 65] = h @ w2a2h
        row[:, 66] = h @ w2a2t
        row[:, 67] = 1.0
        t1_sl.append((h @ a1t + a1b[0]).astype(np.float32))
        tbl_sl.append(row.astype(ed))
    table1a = np.concatenate([t[:RA] for t in tbl_sl], axis=0)
    table1b = np.concatenate([t[RA:] for t in tbl_sl], axis=0)

    def edge_phase(k, tables, t_sl, nw):
        """Returns acc [ROWS, nw] (unnormalized sums + denominator)."""
        acc = np.zeros((ROWS, nw), np.float32)
        relk = REL[k]
        gs_off = meta["gs_off"]
        for g in range(NG):
            S = int(meta["S_g"][g])
            so = int(gs_off[g])
            # gather (all idx positions valid; dummies point at row 0)
            gt = np.zeros((128, S, 128), ed)
            for ci in meta["gcalls"][g]:
                c0, cols, n_idx, base_s, n_slots, is_b, _g = meta["calls"][ci]
                if n_slots == 0:
                    continue
                w16 = eidx[k][:16, c0:c0 + cols]
                flat = w16.T.flatten()[:n_idx].astype(np.int64)
                rows = tables[1 if is_b else 0][flat].astype(ed)
                gt[:, base_s - so:base_s - so + n_slots, :] = np.transpose(
                    rows.reshape(n_slots, 128, 128), (1, 0, 2)
                )
            trow = np.zeros((NG * 128,), np.float32)
            trow[:ROWS] = t_sl
            trowg = trow[g * 128:(g + 1) * 128].astype(ed)  # [128]
            rel_g = relk[:, so:so + S]  # [128, S]
            iota = np.arange(BPB, dtype=np.float32)
            Mx = (rel_g[:, :, None] == iota[None, None, :]).astype(ed)  # [128,S,32]
            # tsel: per (q, segment)
            tsel = np.zeros((128, S, BPB), ed)
            for q in range(GPB):
                tw = trowg[q * BPB:(q + 1) * BPB]
                lo_s0, nlo_q, hi_s0, nhi_q = meta["segments"][g][q]
                for (s0, ns_) in ((lo_s0, nlo_q), (hi_s0, nhi_q)):
                    tsel[:, s0:s0 + ns_, :] = (
                        Mx[:, s0:s0 + ns_, :] * tw[None, None, :]
                    )
            ted = tsel.sum(axis=2, dtype=np.float32).astype(ed)  # [128, S]
            z = (gt[:, :, 0].astype(np.float32) + ted.astype(np.float32))
            zl = np.maximum(z, NEG * z)
            w = np.exp(zl - WSHIFT).astype(ed)
            Wm = (Mx * w[:, :, None]).astype(ed)  # [128, S, 32]
            for q in range(GPB):
                r0 = g * 128 + q * BPB
                a = np.zeros((BPB, nw), np.float32)
                lo_s0, nlo_q, hi_s0, nhi_q = meta["segments"][g][q]
                slots = list(range(lo_s0, lo_s0 + nlo_q)) + list(
                    range(hi_s0, hi_s0 + nhi_q)
                )
                for s in slots:
                    a += (
                        Wm[:, s, :].astype(np.float32).T
                        @ gt[:, s, 1:1 + nw].astype(np.float32)
                    )
                acc[r0:r0 + BPB] = a
        return acc

    # ---- L1 edge phase + L2 table build
    t2_sl, tbl2_sl = [], []
    for k in range(NCORES):
        acc = edge_phase(k, (table1a, table1b), t1_sl[k], NW1)
        den = acc[:, 66:67] + EPS
        h2 = acc[:, 0:64] / den + b2
        u2 = acc[:, 64] / den[:, 0] + u2bias
        t2 = acc[:, 65] / den[:, 0] + t2bias
        row = np.zeros((ROWS, 128), np.float32)
        row[:, 0] = u2
        row[:, 1:65] = h2
        row[:, 65] = 1.0
        t2_sl.append(t2.astype(np.float32))
        tbl2_sl.append(row.astype(ed))
    table2a = np.concatenate([t[:RA] for t in tbl2_sl], axis=0)
    table2b = np.concatenate([t[RA:] for t in tbl2_sl], axis=0)
    host_model.t1_sl = t1_sl
    host_model.t2_sl = t2_sl

    outs = []
    for k in range(NCORES):
        acc = edge_phase(k, (table2a, table2b), t2_sl[k], NW2)
        den = acc[:, 64:65] + EPS
        o2 = acc[:, 0:64] / den
        m = o2.max(axis=1, keepdims=True)
        lse = np.log(np.exp(o2 - m).sum(axis=1, keepdims=True)) + m
        outs.append((o2 - lse)[:NPC])
    return np.concatenate(outs, axis=0).astype(np.float32)


if __name__ == "__main__":
    sys.path.insert(0, os.path.dirname(os.path.abspath(__file__)))
    import reference

    inputs = {k: np.asarray(v) for k, v in reference.setup_inputs().items()}
    expect = np.asarray(reference.reference(**inputs))
    for f16 in (True, False):
        got = host_model(inputs, f16=f16)
        err = np.abs(got - expect)
        rel = err.max() / np.abs(expect).max()
        print(f"host_model f16={f16}: absmax {err.max():.3e} rel {rel:.3e}")


# ------------------------------------------------------------------ bass build
def _patch_tile_drain():
    """This walrus build supports only one sync-wait per SP TPB_CTRL
    instruction; split TileContext's exit drain into single-wait NOPs."""
    import concourse.mybir as mybir
    import concourse.tile as tile
    from concourse.tile import ScopedClock

    if getattr(tile.TileContext, "_drain_split_patched", False):
        return

    def _split(self, tick_clock, wait_clock):
        nop0 = self.nc.sync.nop()
        wait_clock.add_sem_waits(
            nop0.ins, ScopedClock({None: tick_clock.global_clock})
        )
        si = nop0.ins.sync_info
        if si is not None and si.on_wait and len(si.on_wait) > 1:
            waits = list(si.on_wait)
            nop0.ins.sync_info = mybir.SyncInfo(
                on_wait=[waits[0]], on_update=list(si.on_update)
            )
            for w in waits[1:]:
                n = self.nc.sync.nop()
                n.ins.sync_info = mybir.SyncInfo(on_wait=[w], on_update=[])
        self.nc.sync.drain()
        self.nc.all_engine_barrier()
        popped = self.nc._tile_sem_poison_stack.pop()
        assert popped is self._sem_poison
        self.nc.clear_and_free_semaphores(list(self.sems.allocated().values()))
        self.nc.all_engine_barrier()

    tile.TileContext._drain_and_barrier = _split
    tile.TileContext._drain_split_patched = True


def _split_multi_waits(nc):
    """Move extra sync waits onto single-wait NOPs (walrus build limit)."""
    import concourse.mybir as mybir

    ctr = [0]
    for f in nc.m.functions:
        for bb in f.blocks:
            insts = list(bb.instructions)
            out = []
            changed = False
            for ins in insts:
                si = getattr(ins, "sync_info", None)
                if si is not None and si.on_wait and len(si.on_wait) > 1:
                    waits = list(si.on_wait)
                    for w in waits[:-1]:
                        n = mybir.InstNoOp(
                            name=f"splitw-{ctr[0]}", ins=[], outs=[]
                        )
                        ctr[0] += 1
                        n.engine = ins.engine
                        n.sync_info = mybir.SyncInfo(on_wait=[w], on_update=[])
                        nc.register_instruction(n)
                        out.append(n)
                    ins.sync_info = mybir.SyncInfo(
                        on_wait=[waits[-1]], on_update=list(si.on_update)
                    )
                    changed = True
                out.append(ins)
            if changed:
                bb.instructions = out


def build_bass(meta, consts, f16=True):
    import concourse.bass as bass
    import concourse.mybir as mybir
    import concourse.tile as tile
    from concourse.library_config import mlp as mlp_lib
    from concourse.tile_rust import add_dep_helper

    _patch_tile_drain()

    F32 = mybir.dt.float32
    F16 = mybir.dt.float16 if f16 else mybir.dt.float32
    I16 = mybir.dt.int16
    AL = mybir.AluOpType
    AF = mybir.ActivationFunctionType
    AX = mybir.AxisListType

    S_TOT = meta["S_TOT"]
    SMAX = meta["SMAX"]
    gs_off = meta["gs_off"]
    segments = meta["segments"]
    TOTCOLS = meta["TOTCOLS"]

    nc = bass.Bass(
        num_devices=NCORES,
        num_swdge_queues=int(os.environ.get("GNN_NQ", "1")),
        dynamic_dma_scratch_size=int(os.environ.get("GNN_SCR", "65536")),
    )

    x_slT = nc.dram_tensor("x_slT", [IN_F, ROWS], F32, kind="ExternalInput")
    eidx = nc.dram_tensor("eidx", [128, meta["TOTCOLS"]], I16, kind="ExternalInput")
    tgtrel = nc.dram_tensor("tgtrel", [128, S_TOT], F16, kind="ExternalInput")
    out_fin = nc.dram_tensor("out_fin", [ROWS, OUT_F], F32, kind="ExternalOutput")

    def inl(name, arr):
        return nc.inline_tensor(np.ascontiguousarray(arr), name=name)

    np_ed = np.float16 if f16 else np.float32
    c_W1a = inl("W1a", consts["W1"][:128].astype(np.float32))
    c_W1b = inl("W1b", consts["W1"][128:].astype(np.float32))
    c_W2 = inl("W2c", consts["W2"].astype(np.float32))
    c_a1h = inl("a1h", consts["a1h"].reshape(HID_F, 1).astype(np.float32))
    c_a1t = inl("a1t", consts["a1t"].reshape(HID_F, 1).astype(np.float32))
    c_w2a2h = inl("w2a2h", consts["w2a2h"].reshape(HID_F, 1).astype(np.float32))
    c_w2a2t = inl("w2a2t", consts["w2a2t"].reshape(HID_F, 1).astype(np.float32))
    c_b1 = inl("b1c", consts["b1"].reshape(HID_F, 1).astype(np.float32))
    c_b1n = inl("b1n", (-consts["b1"]).reshape(HID_F, 1).astype(np.float32))
    c_b2bc = inl("b2bc", np.tile(consts["b2"].astype(np.float32), (128, 1)))
    c_iota = inl("iota32", np.tile(np.arange(BPB, dtype=np_ed), (128, 1)))
    c_ones1 = inl("ones1", np.ones((1, 128), np.float32))
    c_I128 = inl("I128", np.eye(128, dtype=np.float32))
    c_b1a = inl("b1a", np.full((1, 1), consts["a1b"], np.float32))
    c_nsh = inl("nsh", np.full((128, 1), -WSHIFT, np.float32))
    c_u2b = inl("u2b", np.full((128, 1), consts["u2bias"], np.float32))
    c_t2b = inl("t2b", np.full((128, 1), consts["t2bias"], np.float32))

    h1a_sl = nc.dram_tensor("h1a_sl", [RA, 128], F16)
    h1b_sl = nc.dram_tensor("h1b_sl", [RB, 128], F16)
    h1a_all = nc.dram_tensor("h1a_all", [TBLA, 128], F16, addr_space="Shared")
    h1b_all = nc.dram_tensor("h1b_all", [TBLB, 128], F16, addr_space="Shared")
    h2a_sl = nc.dram_tensor("h2a_sl", [RA, 128], F16)
    h2b_sl = nc.dram_tensor("h2b_sl", [RB, 128], F16)
    h2a_all = nc.dram_tensor("h2a_all", [TBLA, 128], F16, addr_space="Shared")
    h2b_all = nc.dram_tensor("h2b_all", [TBLB, 128], F16, addr_space="Shared")

    def allgather(src, dst):
        import concourse.mybir as _mb
        nc.gpsimd.collective_compute(
            "AllGather",
            _mb.AluOpType.bypass,
            replica_groups=[list(range(NCORES))],
            ins=[src.ap().opt()],
            outs=[dst.ap().opt()],
        )

    with tile.TileContext(nc) as tc:
        import contextlib

        with contextlib.ExitStack() as ctx:
            cpool = ctx.enter_context(tc.tile_pool(name="consts", bufs=1))
            persist = ctx.enter_context(tc.tile_pool(name="persist", bufs=1))
            sb = ctx.enter_context(tc.tile_pool(name="sb", bufs=4))
            gp = ctx.enter_context(tc.tile_pool(name="gp", bufs=4))
            ps = ctx.enter_context(tc.tile_pool(name="ps", bufs=4, space="PSUM"))
            psa = ctx.enter_context(tc.tile_pool(name="psa", bufs=2, space="PSUM"))

            def cload(handle, shape, dtype):
                t = cpool.tile(shape, dtype, tag=handle.name)
                nc.sync.dma_start(out=t[:], in_=handle[:, :])
                return t

            W1a = cload(c_W1a, [128, 128], F32)
            W1b = cload(c_W1b, [128, 128], F32)
            W2 = cload(c_W2, [128, 64], F32)
            a1h = cload(c_a1h, [128, 1], F32)
            a1t = cload(c_a1t, [128, 1], F32)
            w2a2h = cload(c_w2a2h, [128, 1], F32)
            w2a2t = cload(c_w2a2t, [128, 1], F32)
            b1c = cload(c_b1, [128, 1], F32)
            b1n = cload(c_b1n, [128, 1], F32)
            b2bc = cload(c_b2bc, [128, 64], F32)
            iota = cload(c_iota, [128, BPB], F16)
            ones1 = cload(c_ones1, [1, 128], F32)
            I128 = cload(c_I128, [128, 128], F32)
            b1a = cload(c_b1a, [1, 1], F32)
            nsh = cload(c_nsh, [128, 1], F32)
            u2b = cload(c_u2b, [128, 1], F32)
            t2b = cload(c_t2b, [128, 1], F32)

            t1_sb = persist.tile([1, ROWS], F32, tag="t1")
            t2_sb = persist.tile([1, ROWS], F32, tag="t2")
            trowall = persist.tile([128, NG * 128], F16, tag="trowall")
            trb = persist.tile([128, S_TOT], F16, tag="trb")
            nc.sync.dma_start(out=trb[:], in_=tgtrel[:, :])
            eib = persist.tile([128, TOTCOLS], I16, tag="eib")
            nc.sync.dma_start(out=eib[:], in_=eidx[:, :])

            ll = nc.gpsimd.load_library(mlp_lib)
            nidx_regs = {}
            for c in meta["calls"]:
                ni = c[2]
                if ni not in nidx_regs and ni > 0:
                    r = nc.gpsimd.alloc_register(f"nidx_{ni}")
                    nc.gpsimd.reg_mov(r, ni)
                    nidx_regs[ni] = r

            # warm up gather tiles so pad columns never hold NaN bit patterns
            for _ in range(4):
                gt = gp.tile([128, SMAX, 128], F16, tag="g")
                nc.vector.memset(gt[:], 0.0)

            # ---------------- phase 1: table1 rows + t1 ----------------
            for cix in range(NG):
                r0 = cix * 128
                xT0 = sb.tile([128, 128], F32, tag="xT0")
                xT1 = sb.tile([128, 128], F32, tag="xT1")
                nc.sync.dma_start(out=xT0[:], in_=x_slT[0:128, r0:r0 + 128])
                nc.sync.dma_start(out=xT1[:], in_=x_slT[128:256, r0:r0 + 128])
                hTp = ps.tile([128, 128], F32, tag="pp")
                nc.tensor.matmul(hTp[:], lhsT=W1a[:], rhs=xT0[:], start=True, stop=False)
                nc.tensor.matmul(hTp[:], lhsT=W1b[:], rhs=xT1[:], start=False, stop=True)
                ha = sb.tile([128, 128], F32, tag="ha")
                nc.scalar.activation(ha[:], hTp[:], AF.Relu, bias=b1c[:])
                hcn = sb.tile([128, 128], F32, tag="hcn")
                nc.scalar.activation(hcn[:], hTp[:], AF.Relu, bias=b1n[:], scale=-1.0)
                hdx = sb.tile([128, 128], F32, tag="hdx")
                nc.scalar.activation(hdx[:], hcn[:], AF.Exp, scale=-1.0)
                h1T = sb.tile([128, 128], F32, tag="h1T")
                nc.vector.scalar_tensor_tensor(
                    out=h1T[:], in0=hdx[:], scalar=-1.0, in1=ha[:],
                    op0=AL.add, op1=AL.add,
                )
                P = ps.tile([128, NW1], F32, tag="pp")
                nc.tensor.matmul(P[:, 1:65], lhsT=h1T[:], rhs=W2[:], start=True, stop=True)
                nc.tensor.matmul(P[:, 0:1], lhsT=h1T[:], rhs=a1h[:], start=True, stop=True)
                nc.tensor.matmul(P[:, 65:66], lhsT=h1T[:], rhs=w2a2h[:], start=True, stop=True)
                nc.tensor.matmul(P[:, 66:67], lhsT=h1T[:], rhs=w2a2t[:], start=True, stop=True)
                t1p = ps.tile([1, 128], F32, tag="pp")
                nc.tensor.matmul(t1p[:], lhsT=a1t[:], rhs=h1T[:], start=True, stop=True)
                nc.scalar.activation(
                    t1_sb[0:1, r0:r0 + 128], t1p[:], AF.Identity, bias=b1a[:]
                )
                hrow = sb.tile([128, 128], F16, tag="hrow")
                nc.scalar.copy(hrow[:, 0:NW1], P[:])
                nc.vector.memset(hrow[:, NW1:NW1 + 1], 1.0)
                if cix < NGA:
                    nc.sync.dma_start(out=h1a_sl[r0:r0 + 128, :], in_=hrow[:])
                else:
                    nc.sync.dma_start(
                        out=h1b_sl[r0 - RA:r0 - RA + 128, :], in_=hrow[:]
                    )
                if cix == NGA - 1:
                    allgather(h1a_sl, h1a_all)

            allgather(h1b_sl, h1b_all)

            qctr = [0]

            def edge_layer(layer):
                if layer == 1:
                    tables, t_sb, nw = (h1a_all, h1b_all), t1_sb, NW1
                else:
                    tables, t_sb, nw = (h2a_all, h2b_all), t2_sb, NW2

                # trowall: broadcast t values down partitions, fp16
                for g in range(NG):
                    r0 = g * 128
                    trp = ps.tile([128, 128], F32, tag="pp")
                    nc.tensor.matmul(
                        trp[:], lhsT=ones1[:], rhs=t_sb[0:1, r0:r0 + 128],
                        start=True, stop=True,
                    )
                    nc.scalar.copy(trowall[:, r0:r0 + 128], trp[:])

                for g in range(NG):
                    S = int(meta["S_g"][g])
                    so = int(gs_off[g])
                    gt = gp.tile([128, SMAX, 128], F16, tag="g")
                    for ci in meta["gcalls"][g]:
                        (co, cols, n_idx, base_s, n_slots, is_b, _g) = \
                            meta["calls"][ci]
                        if n_slots == 0:
                            continue
                        tbl_ap = tables[1 if is_b else 0][:, :]
                        qctr[0] += 1
                        gi = nc.gpsimd.dma_gather(
                            gt[:, base_s - so:base_s - so + n_slots, :],
                            tbl_ap,
                            eib[:, co:co + cols],
                            num_idxs=n_idx,
                            num_idxs_reg=nidx_regs[n_idx],
                            elem_size=128,
                            single_packet=False,
                            queue_num=qctr[0] % nc.num_swdge_queues,
                        )
                        add_dep_helper(gi.ins, ll.ins)

                    M = sb.tile([128, SMAX, BPB], F16, tag="M")
                    nc.vector.tensor_tensor(
                        out=M[:, 0:S, :],
                        in0=trb[:, so:so + S].to_broadcast([128, S, BPB]),
                        in1=bass.AP(
                            iota[:].tensor, iota[:].offset,
                            [list(iota[:].ap[0]), [0, S], list(iota[:].ap[1])],
                        ),
                        op=AL.is_equal,
                    )
                    tsel = sb.tile([128, SMAX, BPB], F16, tag="tsel")
                    trg = trowall[:, g * 128:(g + 1) * 128]
                    for q in range(GPB):
                        tq = trg[:, q * BPB:(q + 1) * BPB]
                        lo_s0, nlo_q, hi_s0, nhi_q = segments[g][q]
                        for (s0_, ns_) in ((lo_s0, nlo_q), (hi_s0, nhi_q)):
                            if ns_ == 0:
                                continue
                            nc.vector.tensor_tensor(
                                out=tsel[:, s0_:s0_ + ns_, :],
                                in0=M[:, s0_:s0_ + ns_, :],
                                in1=bass.AP(
                                    tq.tensor, tq.offset,
                                    [list(tq.ap[0]), [0, ns_], list(tq.ap[1])],
                                ),
                                op=AL.mult,
                            )
                    ted = sb.tile([128, SMAX], F16, tag="ted")
                    with nc.allow_low_precision(
                        reason="one nonzero per segment; fp16 exact"
                    ):
                        nc.vector.tensor_reduce(
                            out=ted[:, 0:S], in_=tsel[:, 0:S, :],
                            axis=AX.X, op=AL.add,
                        )
                    # z = g0 + ted ; zl = max(z, 0.2 z) ; w = exp(zl - 8)
                    g0 = bass.AP(
                        gt[:].tensor, gt[:].offset,
                        [list(gt[:].ap[0]), [128, S]],
                    )
                    z = sb.tile([128, SMAX], F16, tag="z")
                    nc.vector.tensor_tensor(
                        out=z[:, 0:S], in0=g0, in1=ted[:, 0:S], op=AL.add
                    )
                    zl = sb.tile([128, SMAX], F16, tag="zl")
                    nc.vector.scalar_tensor_tensor(
                        out=zl[:, 0:S], in0=z[:, 0:S], scalar=NEG,
                        in1=z[:, 0:S], op0=AL.mult, op1=AL.max,
                    )
                    w = sb.tile([128, SMAX], F16, tag="w")
                    nc.scalar.activation(w[:, 0:S], zl[:, 0:S], AF.Exp, bias=nsh[:])
                    Wm = sb.tile([128, SMAX, BPB], F16, tag="Wm")
                    nc.vector.tensor_tensor(
                        out=Wm[:, 0:S, :],
                        in0=M[:, 0:S, :],
                        in1=w[:, 0:S].to_broadcast([128, S, BPB]),
                        op=AL.mult,
                    )

                    acc01 = psa.tile([64, nw], F32, tag="acc01")
                    acc23 = psa.tile([64, nw], F32, tag="acc23")
                    accs = [acc01, acc23]
                    for q in range(GPB):
                        acc = accs[q // 2]
                        qq = q % 2
                        lo_s0, nlo_q, hi_s0, nhi_q = segments[g][q]
                        slots = list(range(lo_s0, lo_s0 + nlo_q)) + list(
                            range(hi_s0, hi_s0 + nhi_q)
                        )
                        for si, sl in enumerate(slots):
                            nc.tensor.matmul(
                                acc[qq * BPB:(qq + 1) * BPB, :],
                                lhsT=Wm[:, sl, :],
                                rhs=gt[:, sl, 1:1 + nw],
                                start=(si == 0), stop=(si == len(slots) - 1),
                                skip_group_check=True,
                            )

                    for half, acc in enumerate(accs):
                        r0 = g * 128 + half * 64
                        den = sb.tile([64, 1], F32, tag="den")
                        nc.vector.tensor_scalar_add(den[:], acc[:, nw - 1:nw], EPS)
                        rec = sb.tile([64, 1], F32, tag="rec")
                        nc.vector.reciprocal(rec[:], den[:])
                        if layer == 1:
                            h2p = sb.tile([64, 64], F32, tag="h2p")
                            nc.vector.tensor_tensor(
                                out=h2p[:], in0=acc[:, 0:64],
                                in1=rec[:].to_broadcast([64, 64]), op=AL.mult,
                            )
                            h2 = sb.tile([64, 64], F32, tag="h2")
                            nc.vector.tensor_tensor(
                                out=h2[:], in0=h2p[:], in1=b2bc[0:64, :], op=AL.add
                            )
                            hrow2 = sb.tile([64, 128], F16, tag="hrow2")
                            nc.scalar.activation(
                                hrow2[:, 0:1], acc[:, 64:65], AF.Identity,
                                scale=rec[:], bias=u2b[0:64, :],
                            )
                            nc.scalar.copy(hrow2[:, 1:65], h2[:])
                            nc.vector.memset(hrow2[:, 65:66], 1.0)
                            t2c = sb.tile([64, 1], F32, tag="t2c")
                            nc.scalar.activation(
                                t2c[:], acc[:, 65:66], AF.Identity,
                                scale=rec[:], bias=t2b[0:64, :],
                            )
                            t2pp = ps.tile([1, 64], F32, tag="pp")
                            nc.tensor.transpose(t2pp[:], t2c[:], I128[0:64, 0:64])
                            nc.scalar.copy(t2_sb[0:1, r0:r0 + 64], t2pp[:])
                            if r0 < RA:
                                nc.sync.dma_start(
                                    out=h2a_sl[r0:r0 + 64, :], in_=hrow2[:]
                                )
                            else:
                                nc.sync.dma_start(
                                    out=h2b_sl[r0 - RA:r0 - RA + 64, :],
                                    in_=hrow2[:],
                                )
                        else:
                            o2 = sb.tile([64, 64], F32, tag="o2")
                            nc.vector.tensor_tensor(
                                out=o2[:], in0=acc[:, 0:64],
                                in1=rec[:].to_broadcast([64, 64]), op=AL.mult,
                            )
                            mx = sb.tile([64, 1], F32, tag="mx")
                            nc.vector.tensor_reduce(
                                out=mx[:], in_=o2[:], axis=AX.X, op=AL.max
                            )
                            mneg = sb.tile([64, 1], F32, tag="mneg")
                            nc.vector.tensor_scalar_mul(mneg[:], mx[:], -1.0)
                            ex = sb.tile([64, 64], F32, tag="ex")
                            nc.scalar.activation(ex[:], o2[:], AF.Exp, bias=mneg[:])
                            sm = sb.tile([64, 1], F32, tag="sm")
                            nc.vector.tensor_reduce(
                                out=sm[:], in_=ex[:], axis=AX.X, op=AL.add
                            )
                            ln = sb.tile([64, 1], F32, tag="ln")
                            nc.scalar.activation(ln[:], sm[:], AF.Ln)
                            mml = sb.tile([64, 1], F32, tag="mml")
                            nc.vector.tensor_tensor(
                                out=mml[:], in0=mx[:], in1=ln[:], op=AL.add
                            )
                            res = sb.tile([64, 64], F32, tag="res")
                            nc.vector.tensor_tensor(
                                out=res[:], in0=o2[:],
                                in1=mml[:].to_broadcast([64, 64]), op=AL.subtract,
                            )
                            nc.sync.dma_start(
                                out=out_fin[r0:r0 + 64, :], in_=res[:]
                            )
                    if layer == 1 and g == NGA - 1:
                        allgather(h2a_sl, h2a_all)

            edge_layer(1)
            allgather(h2b_sl, h2b_all)
            edge_layer(2)

    return nc


def kernel(**inputs):
    from concourse.bass_utils import run_bass_kernel_spmd
    from concourse.library_overlay import lower_extended_insts

    x = np.asarray(inputs["x"], np.float32)
    ei = np.asarray(inputs["edge_index"])
    meta, eidx, REL = prep_structures(ei)
    W2 = np.asarray(inputs["W2"], np.float32)
    b2 = np.asarray(inputs["b2"], np.float32)
    a1w = np.asarray(inputs["a1_w"], np.float32)
    a2w = np.asarray(inputs["a2_w"], np.float32)
    consts = dict(
        W1=np.asarray(inputs["W1"], np.float32),
        b1=np.asarray(inputs["b1"], np.float32),
        W2=W2,
        b2=b2,
        a1h=a1w[:HID_F], a1t=a1w[HID_F:],
        a1b=float(np.asarray(inputs["a1_b"], np.float32)[0]),
        w2a2h=W2 @ a2w[:OUT_F],
        w2a2t=W2 @ a2w[OUT_F:],
        u2bias=float(a2w[:OUT_F] @ b2),
        t2bias=float(a2w[OUT_F:] @ b2 + np.asarray(inputs["a2_b"], np.float32)[0]),
    )
    f16 = os.environ.get("GNN_F32", "0") != "1"
    nc = build_bass(meta, consts, f16=f16)
    _split_multi_waits(nc)
    lower_extended_insts(nc)

    np_ed = np.float16 if f16 else np.float32
    in_maps = []
    for k in range(NCORES):
        xs = np.zeros((ROWS, IN_F), np.float32)
        xs[:NPC] = x[k * NPC:(k + 1) * NPC]
        in_maps.append(
            {
                "x_slT": np.ascontiguousarray(xs.T),
                "eidx": np.ascontiguousarray(eidx[k]),
                "tgtrel": np.ascontiguousarray(REL[k].astype(np_ed)),
            }
        )

    trace = os.environ.get("GNN_TRACE", "0") == "1"
    if trace:
        try:
            import types
            from trn_agent_boot.trn_boot import _ntff_profile_via_ctypes
            _h = _ntff_profile_via_ctypes("/opt/axon/libaxon_pjrt.so")
            m = types.ModuleType("antenv.axon_hooks")
            m.get_axon_ntff_profile_hook = lambda: _h
            sys.modules["antenv.axon_hooks"] = m
        except Exception as e:
            print("profile hook setup failed:", e)
            trace = False
    res = run_bass_kernel_spmd(
        nc, in_maps, core_ids=list(range(NCORES)), trace=trace
    )
    kernel.last_results = res
    out = np.concatenate(
        [res.results[k]["out_fin"][:NPC] for k in range(NCORES)], axis=0
    )
    return out.astype(np.float32)
